# revision 13
# baseline (speedup 1.0000x reference)
"""ActSpanDecoder step on 8 Trainium2 NeuronCores.

Strategy: data-parallel over batch (16 rows/core), identical SPMD program on
all 8 cores (no collectives). The [B,Tb,V+Tb] one-hot scatter matrix is never
read: the copy-score scatter is reconstructed on-device from bspn_nounk via a
factorized (25x128) one-hot matmul, which accumulates duplicate indices in
PSUM exactly like the reference einsum. Matmul inputs are bf16 (fp32
accumulate); softmax / log-space math stays fp32.
"""

import sys

sys.path.insert(0, "/opt/trn_rl_repo")

import numpy as np
import ml_dtypes

import concourse.bass as bass
import concourse.tile as tile
from concourse import bacc, mybir
from concourse.bass_utils import run_bass_kernel_spmd
from concourse.masks import make_identity

BF16 = mybir.dt.bfloat16
F32 = mybir.dt.float32
F32R = mybir.dt.float32r
I32 = mybir.dt.int32
AF = mybir.ActivationFunctionType
ALU = mybir.AluOpType
AX = mybir.AxisListType

NCORES = 8
B, Tu, Tb, Tp = 128, 256, 128, 64
BL = B // NCORES  # 16
H, E, V, VOOV, PTR = 512, 512, 3000, 3400, 32
NEG = -1e20
HC = 4
NI = 25          # 25*128 = 3200 >= V+Tb = 3128
NIP = 26         # padded matmul M (even partition count for PSUM reads)
VP = NI * 128
NOOV = VOOV - V  # 400
VTB = V + Tb

_nbf = ml_dtypes.bfloat16

DEBUG = False
PHASE = 99  # bisection: stop after phase N

ENCS = [("u", Tu), ("b", Tb), ("p", Tp)]


def _chunked(a):
    """[512, X] -> [128, 4*X] with h-chunk c at cols [c*X:(c+1)*X]."""
    h, x = a.shape
    assert h == 4 * 128
    return np.ascontiguousarray(
        a.reshape(4, 128, x).transpose(1, 0, 2).reshape(128, 4 * x)
    )


def build_program():
    nc = bacc.Bacc("TRN2", target_bir_lowering=False, debug=False,
                   num_devices=NCORES)

    def din(name, shape, dt=BF16):
        return nc.dram_tensor(name, list(shape), dt, kind="ExternalInput").ap()

    ins = {}
    for name, shape, dt in [
        ("uT", (128, 4 * BL * Tu), BF16), ("bT", (128, 4 * BL * Tb), BF16),
        ("pT", (128, 4 * BL * Tp), BF16),
        ("uN", (BL * Tu, 512), BF16), ("bN", (BL * Tb, 512), BF16),
        ("pN", (BL * Tp, 512), BF16),
        ("w1t", (128, 4 * 512), BF16), ("w2t", (128, 4 * 512), BF16),
        ("wct", (128, 4 * 512), BF16), ("wgent", (128, 4 * 3000), BF16),
        ("wihrz", (128, 17 * 1024), BF16), ("wihn", (128, 17 * 512), BF16),
        ("whht", (128, 4 * 1536), BF16),
        ("attnb", (1, 512), BF16), ("wgenb", (1, 3000), BF16),
        ("gbrz", (1, 1024), BF16), ("gbinn", (1, 512), BF16),
        ("gbhn", (1, 512), BF16),
        ("h0t", (128, 4 * BL), BF16), ("h0f", (BL, 512), F32),
        ("embt", (128, 4 * BL), BF16), ("dbt", (PTR, BL), BF16),
        ("vq16", (128, 4 * BL), BF16), ("wcb", (128, 4), F32),
        ("msu", (BL, BL * Tu), F32), ("msb", (BL, BL * Tb), F32),
        ("msp", (BL, BL * Tp), F32), ("mbT", (128, BL), F32),
        ("colhiT", (128, BL), F32), ("colloT", (128, BL), F32),
        ("khiT", (128, BL), F32), ("kloT", (128, BL), F32),
        ("oovf", (BL, Tb), F32),
    ]:
        ins[name] = din(name, shape, dt)

    out_ap = nc.dram_tensor("out", [BL, VOOV], F32, kind="ExternalOutput").ap()

    dbg = {}
    if DEBUG:
        for name, shape in [
            ("dq", (BL, 512)), ("dctxu", (BL, 512)), ("dctxb", (BL, 512)),
            ("dctxp", (BL, 512)), ("dhnew", (BL, 512)), ("dcprawT", (128, BL)),
            ("dcps", (BL, VP)), ("dlse", (BL, 1)),
        ]:
            dbg[name] = nc.dram_tensor(name, list(shape), F32,
                                       kind="ExternalOutput").ap()

    with tile.TileContext(nc) as tc:
        _emit(tc, nc, ins, out_ap, dbg)

    nc.compile()
    return nc


def _emit(tc, nc, ins, out_ap, dbg):
    from contextlib import ExitStack

    ctx = ExitStack()
    with ctx:
        konst = ctx.enter_context(tc.tile_pool(name="konst", bufs=1))
        sb = ctx.enter_context(tc.tile_pool(name="sb", bufs=1))
        enc_pool = ctx.enter_context(tc.tile_pool(name="encn", bufs=4))
        et_pool = ctx.enter_context(tc.tile_pool(name="et", bufs=8))
        wstream = ctx.enter_context(tc.tile_pool(name="wstream", bufs=3))
        pp_s = ctx.enter_context(tc.tile_pool(name="pps", bufs=2, space="PSUM"))
        pp_sm = ctx.enter_context(tc.tile_pool(name="ppsm", bufs=2, space="PSUM"))
        pp_tr = ctx.enter_context(tc.tile_pool(name="pptr", bufs=1, space="PSUM"))

        # ---- constants ----
        ident = konst.tile([128, 128], F32, tag="ident")
        make_identity(nc, ident[:])
        ones16 = konst.tile([1, BL], BF16, tag="ones16")
        nc.vector.memset(ones16[:], 1.0)

        def load(name, dt=BF16, pool=konst):
            ap = ins[name]
            t = pool.tile(list(ap.shape), dt, tag="k_" + name)
            nc.sync.dma_start(t[:], ap[:])
            return t

        w1t_s = load("w1t")
        w2t_s = load("w2t")
        wct_s = load("wct")
        attnb_s = load("attnb")
        wgenb_s = load("wgenb")
        gbrz_s = load("gbrz")
        gbinn_s = load("gbinn")
        gbhn_s = load("gbhn")
        h0t_s = load("h0t")
        h0f_s = load("h0f", F32)
        embt_s = load("embt")
        dbt_s = load("dbt")
        vq16_s = load("vq16")
        wcb_s = load("wcb", F32)
        maskst_s = {"u": load("msu", F32), "b": load("msb", F32),
                    "p": load("msp", F32)}
        maskbT_s = load("mbT", F32)
        colhiT_s = load("colhiT", F32)
        colloT_s = load("colloT", F32)
        khiT_s = load("khiT", F32)
        kloT_s = load("kloT", F32)
        oovf_s = load("oovf", F32)
        encT = {"u": ins["uT"], "b": ins["bT"], "p": ins["pT"]}
        encN = {"u": ins["uN"], "b": ins["bN"], "p": ins["pN"]}
        etin_pool = ctx.enter_context(tc.tile_pool(name="etin", bufs=8))

        iota25i = konst.tile([128, NIP], I32, tag="iota25i")
        nc.gpsimd.iota(iota25i[:], pattern=[[1, NIP]], base=0,
                       channel_multiplier=0)
        iota25f = konst.tile([128, NIP], F32, tag="iota25f")
        nc.vector.tensor_copy(iota25f[:], iota25i[:])
        iota128i = konst.tile([128, 128], I32, tag="iota128i")
        nc.gpsimd.iota(iota128i[:], pattern=[[1, 128]], base=0,
                       channel_multiplier=0)
        iota128f = konst.tile([128, 128], F32, tag="iota128f")
        nc.vector.tensor_copy(iota128f[:], iota128i[:])
        iota4f = konst.tile([128, 4], F32, tag="iota4f")
        nc.vector.tensor_copy(iota4f[:], iota128i[:, :4])


        def _pad_out():
            zout = sb.tile([BL, VOOV], F32, tag="zout")
            nc.vector.memset(zout[:], 0.0)
            nc.sync.dma_start(out_ap[:], zout[:])

        def transpose_cols(src_ap, nf):
            """src [BL, nf] -> psum [nf, BL] f32 (nf <= 128)."""
            tp = pp_tr.tile([128, BL], F32, tag="tp")
            nc.tensor.transpose(tp[:nf, :BL], src_ap, ident[:BL, :BL])
            return tp

        # ---- q = h0 @ W1.T + attn_b ----
        qp = pp_sm.tile([NI, 512], F32, tag="ps_sm")
        for hc in range(HC):
            nc.tensor.matmul(qp[:BL, :512],
                             lhsT=h0t_s[:, hc * BL:(hc + 1) * BL],
                             rhs=w1t_s[:, hc * 512:(hc + 1) * 512],
                             start=(hc == 0), stop=False)
        nc.tensor.matmul(qp[:BL, :512], lhsT=ones16[:1, :], rhs=attnb_s[:1, :],
                         start=False, stop=True)
        q_sb = konst.tile([BL, 512], F32, tag="q_sb")
        nc.vector.tensor_copy(q_sb[:], qp[:BL, :512])
        if DEBUG:
            nc.sync.dma_start(dbg["dq"][:], q_sb[:])
        qt_sb = konst.tile([128, 4 * BL], F32, tag="qt_sb")
        for hc in range(HC):
            tp = transpose_cols(q_sb[:BL, hc * 128:(hc + 1) * 128], 128)
            nc.vector.tensor_copy(qt_sb[:, hc * BL:(hc + 1) * BL],
                                  tp[:128, :BL])

        if PHASE < 2:
            _pad_out()
            return
        # ---- attention over the three encoders ----
        ctx_sb = {}
        encs_run = ENCS[:1] if PHASE == 2 else ENCS
        for key, T in encs_run:
            cols = BL * T
            ntiles = cols // 512
            nseg = 512 // T
            eT = encT[key]
            nchunks = cols // 128
            abig = sb.tile([128, nchunks * BL], BF16, tag="abig_" + key)
            nc.vector.memset(abig[:], 0.0)
            asump = sb.tile([BL, BL], F32, tag="asump_" + key)
            for nt in range(ntiles):
                etin = []
                for hic in range(HC):
                    ei = etin_pool.tile([128, 512], BF16, tag="etin")
                    nc.sync.dma_start(ei[:], eT[:, hic * cols + nt * 512:
                                                hic * cols + (nt + 1) * 512])
                    etin.append(ei)
                ets = []
                for hoc in range(HC):
                    ps = pp_s.tile([128, 512], F32, tag="ps_s")
                    for hic in range(HC):
                        nc.tensor.matmul(
                            ps[:],
                            lhsT=w2t_s[:, hic * 512 + hoc * 128:
                                       hic * 512 + (hoc + 1) * 128],
                            rhs=etin[hic][:],
                            start=(hic == 0), stop=(hic == HC - 1))
                    et = et_pool.tile([128, 512], BF16, tag="et")
                    for s in range(nseg):
                        b = nt * nseg + s
                        nc.scalar.activation(
                            et[:, s * T:(s + 1) * T], ps[:, s * T:(s + 1) * T],
                            AF.Tanh,
                            bias=qt_sb[:, hoc * BL + b:hoc * BL + b + 1])
                    ets.append(et)
                strip = pp_sm.tile([NI, 512], F32, tag="ps_sm")
                for hoc in range(HC):
                    nc.tensor.matmul(strip[:BL, :512],
                                     lhsT=vq16_s[:, hoc * BL:(hoc + 1) * BL],
                                     rhs=ets[hoc][:], start=(hoc == 0),
                                     stop=(hoc == HC - 1))
                # masked scores in replicated-row strip layout
                astr = sb.tile([BL, 512], F32, tag="astr")
                nc.vector.tensor_tensor(
                    astr[:], strip[:BL, :512],
                    maskst_s[key][:, nt * 512:(nt + 1) * 512], op=ALU.add)
                aex = sb.tile([BL, 512], F32, tag="aex")
                for s in range(nseg):
                    b = nt * nseg + s
                    nc.scalar.activation(aex[:, s * T:(s + 1) * T],
                                         astr[:, s * T:(s + 1) * T], AF.Exp,
                                         accum_out=asump[:, b:b + 1])
                # A^T columns via PE transpose of 128-col blocks
                for blk in range(4):
                    tp = transpose_cols(aex[:BL, blk * 128:(blk + 1) * 128],
                                        128)
                    gtok = nt * 512 + blk * 128  # global (b,t) token index
                    c = gtok // 128
                    if T >= 128:
                        b = gtok // T
                        nc.vector.tensor_copy(
                            abig[:128, c * BL + b:c * BL + b + 1],
                            tp[:128, 0:1])
                    else:  # pv: the 128-token chunk spans two b rows
                        for h in range(2):
                            b = (gtok + h * 64) // T
                            nc.vector.tensor_copy(
                                abig[h * 64:(h + 1) * 64,
                                     c * BL + b:c * BL + b + 1],
                                tp[h * 64:(h + 1) * 64, 0:1])
            # per-b softmax sums: diagonal of asump via identity-masked reduce
            asum = sb.tile([BL, 1], F32, tag="asum_" + key)
            djunk = sb.tile([BL, BL], F32, tag="djunk")
            nc.vector.tensor_tensor(djunk[:], asump[:], ident[:BL, :BL],
                                    op=ALU.mult)
            nc.vector.tensor_reduce(asum[:], djunk[:], axis=AX.X, op=ALU.add)
            rec = sb.tile([BL, 1], F32, tag="rec_" + key)
            nc.vector.reciprocal(rec[:], asum[:])
            # ctx = (A @ enc) / sum
            ctxp = pp_sm.tile([NI, 512], F32, tag="ps_sm")
            for c in range(nchunks):
                encn = enc_pool.tile([128, 512], BF16, tag="encn")
                nc.sync.dma_start(encn[:], encN[key][c * 128:(c + 1) * 128, :])
                nc.tensor.matmul(ctxp[:BL, :512],
                                 lhsT=abig[:128, c * BL:(c + 1) * BL],
                                 rhs=encn[:], start=(c == 0),
                                 stop=(c == nchunks - 1))
            cx = konst.tile([BL, 512], F32, tag="ctx_" + key)
            nc.vector.tensor_scalar(out=cx[:], in0=ctxp[:BL, :512],
                                    scalar1=rec[:, :1], scalar2=None,
                                    op0=ALU.mult)
            ctx_sb[key] = cx
            if DEBUG:
                nc.sync.dma_start(dbg["dctx" + key][:], cx[:])

        if PHASE < 4:
            _pad_out()
            return
        # ---- xT assembly: emb | ctx_u | ctx_b | ctx_p | db ----
        xT = konst.tile([128, 17 * BL], BF16, tag="xT")
        nc.vector.memset(xT[:], 0.0)
        nc.vector.tensor_copy(xT[:, 0:4 * BL], embt_s[:])
        for i, key in enumerate(["u", "b", "p"]):
            for hc in range(HC):
                tp = transpose_cols(ctx_sb[key][:BL, hc * 128:(hc + 1) * 128],
                                    128)
                col = (4 + 4 * i + hc) * BL
                nc.vector.tensor_copy(xT[:, col:col + BL], tp[:128, :BL])
        nc.vector.tensor_copy(xT[:PTR, 16 * BL:17 * BL], dbt_s[:])

        if PHASE < 5:
            _pad_out()
            return
        # ---- GRU (pass 1: r,z with ih+hh fused in PSUM) ----
        ps_r = pp_sm.tile([NI, 512], F32, tag="ps_sm")
        ps_z = pp_sm.tile([NI, 512], F32, tag="ps_sm")
        for k in range(17):
            wrz = wstream.tile([128, 1024], BF16, tag="wrz")
            nc.sync.dma_start(wrz[:], ins["wihrz"][:, k * 1024:(k + 1) * 1024])
            lhs = xT[:, k * BL:(k + 1) * BL]
            nc.tensor.matmul(ps_r[:BL, :512], lhsT=lhs, rhs=wrz[:, 0:512],
                             start=(k == 0), stop=False)
            nc.tensor.matmul(ps_z[:BL, :512], lhsT=lhs, rhs=wrz[:, 512:1024],
                             start=(k == 0), stop=False)
        for hc in range(HC):
            whrz = wstream.tile([128, 1024], BF16, tag="wrz")
            nc.sync.dma_start(whrz[:],
                              ins["whht"][:, hc * 1536:hc * 1536 + 1024])
            lhs = h0t_s[:, hc * BL:(hc + 1) * BL]
            nc.tensor.matmul(ps_r[:BL, :512], lhsT=lhs, rhs=whrz[:, 0:512],
                             start=False, stop=False)
            nc.tensor.matmul(ps_z[:BL, :512], lhsT=lhs, rhs=whrz[:, 512:1024],
                             start=False, stop=False)
        nc.tensor.matmul(ps_r[:BL, :512], lhsT=ones16[:1, :],
                         rhs=gbrz_s[:1, 0:512], start=False, stop=True)
        nc.tensor.matmul(ps_z[:BL, :512], lhsT=ones16[:1, :],
                         rhs=gbrz_s[:1, 512:1024], start=False, stop=True)
        r_sb = sb.tile([BL, 512], F32, tag="r_sb")
        z_sb = sb.tile([BL, 512], F32, tag="z_sb")
        nc.scalar.activation(r_sb[:], ps_r[:BL, :512], AF.Sigmoid)
        nc.scalar.activation(z_sb[:], ps_z[:BL, :512], AF.Sigmoid)

        # ---- GRU (pass 2: inn, hn) ----
        ps_inn = pp_sm.tile([NI, 512], F32, tag="ps_sm")
        ps_hn = pp_sm.tile([NI, 512], F32, tag="ps_sm")
        for k in range(17):
            wn = wstream.tile([128, 512], BF16, tag="wn")
            nc.sync.dma_start(wn[:], ins["wihn"][:, k * 512:(k + 1) * 512])
            nc.tensor.matmul(ps_inn[:BL, :512], lhsT=xT[:, k * BL:(k + 1) * BL],
                             rhs=wn[:], start=(k == 0), stop=False)
        nc.tensor.matmul(ps_inn[:BL, :512], lhsT=ones16[:1, :],
                         rhs=gbinn_s[:1, :], start=False, stop=True)
        for hc in range(HC):
            whn = wstream.tile([128, 512], BF16, tag="wn")
            nc.sync.dma_start(whn[:],
                              ins["whht"][:, hc * 1536 + 1024:hc * 1536 + 1536])
            nc.tensor.matmul(ps_hn[:BL, :512],
                             lhsT=h0t_s[:, hc * BL:(hc + 1) * BL],
                             rhs=whn[:], start=(hc == 0), stop=False)
        nc.tensor.matmul(ps_hn[:BL, :512], lhsT=ones16[:1, :],
                         rhs=gbhn_s[:1, :], start=False, stop=True)

        rhn = sb.tile([BL, 512], F32, tag="rhn")
        nc.vector.tensor_tensor(rhn[:], r_sb[:], ps_hn[:BL, :512], op=ALU.mult)
        npre = sb.tile([BL, 512], F32, tag="npre")
        nc.vector.tensor_tensor(npre[:], rhn[:], ps_inn[:BL, :512], op=ALU.add)
        n_sb = sb.tile([BL, 512], F32, tag="n_sb")
        nc.scalar.activation(n_sb[:], npre[:], AF.Tanh)
        t1 = sb.tile([BL, 512], F32, tag="rhn")
        nc.vector.tensor_tensor(t1[:], h0f_s[:], n_sb[:], op=ALU.subtract)
        t2 = sb.tile([BL, 512], F32, tag="npre")
        nc.vector.tensor_tensor(t2[:], z_sb[:], t1[:], op=ALU.mult)
        hnew = konst.tile([BL, 512], F32, tag="hnew")
        nc.vector.tensor_tensor(hnew[:], t2[:], n_sb[:], op=ALU.add)
        if DEBUG:
            nc.sync.dma_start(dbg["dhnew"][:], hnew[:])
        hnT = konst.tile([128, 4 * BL], BF16, tag="hnT")
        for hc in range(HC):
            tp = transpose_cols(hnew[:BL, hc * 128:(hc + 1) * 128], 128)
            nc.vector.tensor_copy(hnT[:, hc * BL:(hc + 1) * BL], tp[:128, :BL])

        if PHASE < 6:
            _pad_out()
            return
        # ---- copy scores: cp_raw[b,t] = tanh(bspn Wc.T + bc) . hnew + mask ----
        bcols = BL * Tb
        cprawT = konst.tile([128, BL], F32, tag="cprawT")
        for nt in range(4):
            etin = []
            for hic in range(HC):
                ei = etin_pool.tile([128, 512], BF16, tag="etin")
                nc.sync.dma_start(ei[:], encT["b"][:, hic * bcols + nt * 512:
                                                   hic * bcols + (nt + 1) * 512])
                etin.append(ei)
            cpts = []
            for hoc in range(HC):
                ps = pp_s.tile([128, 512], F32, tag="ps_s")
                for hic in range(HC):
                    nc.tensor.matmul(
                        ps[:],
                        lhsT=wct_s[:, hic * 512 + hoc * 128:
                                   hic * 512 + (hoc + 1) * 128],
                        rhs=etin[hic][:],
                        start=(hic == 0), stop=(hic == HC - 1))
                cpt = et_pool.tile([128, 512], BF16, tag="et")
                nc.scalar.activation(cpt[:], ps[:], AF.Tanh,
                                     bias=wcb_s[:, hoc:hoc + 1])
                cpts.append(cpt)
            pscr = pp_sm.tile([NI, 512], F32, tag="ps_sm")
            for hoc in range(HC):
                nc.tensor.matmul(pscr[:BL, :512],
                                 lhsT=hnT[:, hoc * BL:(hoc + 1) * BL],
                                 rhs=cpts[hoc][:], start=(hoc == 0),
                                 stop=(hoc == HC - 1))
            stsb = sb.tile([BL, 512], F32, tag="stripsb")
            nc.any.tensor_copy(stsb[:], pscr[:BL, :512])
            for s in range(4):
                b = nt * 4 + s
                tpc = transpose_cols(stsb[:BL, s * 128:(s + 1) * 128], 128)
                nc.vector.tensor_copy(cprawT[:, b:b + 1], tpc[:128, b:b + 1])
        nc.vector.tensor_tensor(cprawT[:], cprawT[:], maskbT_s[:], op=ALU.add)
        if DEBUG:
            nc.sync.dma_start(dbg["dcprawT"][:], cprawT[:])

        if PHASE < 7:
            _pad_out()
            return
        # ---- factorized scatter: cps[col[t]] += cp_raw[t] ----
        scat_sb = konst.tile([26, 4 * 512], F32, tag="scat_sb")
        for g in range(4):
            psc = pp_sm.tile([26, 512], F32, tag="ps_sm")
            for s in range(4):
                b = g * 4 + s
                m1 = sb.tile([128, NIP], F32R, tag="m1")
                nc.vector.scalar_tensor_tensor(
                    out=m1[:], in0=iota25f[:], scalar=colhiT_s[:, b:b + 1],
                    in1=cprawT[:, b:b + 1].to_broadcast([128, NIP]),
                    op0=ALU.is_equal, op1=ALU.mult)
                lo = sb.tile([128, 128], F32R, tag="lo")
                nc.vector.tensor_scalar(
                    out=lo[:], in0=iota128f[:], scalar1=colloT_s[:, b:b + 1],
                    scalar2=None, op0=ALU.is_equal)
                nc.tensor.matmul(psc[:NIP, s * 128:(s + 1) * 128],
                                 lhsT=m1[:], rhs=lo[:],
                                 start=True, stop=True)
            nc.vector.tensor_copy(scat_sb[:26, g * 512:(g + 1) * 512],
                                  psc[:26, :512])
        cps_flat = konst.tile([BL, VP], F32, tag="cps_flat")
        for b in range(BL):
            g, s = b // 4, b % 4
            nc.sync.dma_start(
                cps_flat[b:b + 1, :],
                scat_sb[:NI, g * 512 + s * 128:g * 512 + (s + 1) * 128])
        if DEBUG:
            nc.sync.dma_start(dbg["dcps"][:], cps_flat[:])
        e_cps = konst.tile([BL, VTB], F32, tag="e_cps")
        scs = sb.tile([BL, 1], F32, tag="scs")
        nc.scalar.activation(e_cps[:], cps_flat[:BL, :VTB], AF.Exp,
                             accum_out=scs[:])

        if PHASE < 8:
            _pad_out()
            return
        # ---- gen = exp(hnew @ Wgen.T + b) ----
        e_gen = konst.tile([BL, 3000], F32, tag="e_gen")
        sgp = sb.tile([BL, 6], F32, tag="sgp")
        nts = [512] * 5 + [440]
        for i, n in enumerate(nts):
            off = i * 512
            pg = pp_sm.tile([NI, 512], F32, tag="ps_sm")
            for hc in range(HC):
                wg = wstream.tile([128, 512], BF16, tag="wn")
                nc.sync.dma_start(wg[:, :n],
                                  ins["wgent"][:, hc * 3000 + off:
                                               hc * 3000 + off + n])
                nc.tensor.matmul(
                    pg[:BL, :n], lhsT=hnT[:, hc * BL:(hc + 1) * BL],
                    rhs=wg[:, :n], start=(hc == 0), stop=False)
            nc.tensor.matmul(pg[:BL, :n], lhsT=ones16[:1, :],
                             rhs=wgenb_s[:1, off:off + n], start=False,
                             stop=True)
            nc.scalar.activation(e_gen[:, off:off + n], pg[:BL, :n], AF.Exp,
                                 accum_out=sgp[:, i:i + 1])
        sg = sb.tile([BL, 1], F32, tag="sg")
        nc.vector.tensor_reduce(sg[:], sgp[:], axis=AX.X, op=ALU.add)

        # ---- normalization ----
        stot = sb.tile([BL, 1], F32, tag="stot")
        nc.vector.tensor_tensor(stot[:], sg[:], scs[:], op=ALU.add)
        lse = sb.tile([BL, 1], F32, tag="lse")
        nc.scalar.activation(lse[:], stot[:], AF.Ln)
        rtot = sb.tile([BL, 1], F32, tag="rtot")
        nc.vector.reciprocal(rtot[:], stot[:])
        if DEBUG:
            nc.sync.dma_start(dbg["dlse"][:], lse[:])

        # total[:, :V] = ln(e_gen + e_cps[:, :V]) - lse
        nc.vector.tensor_tensor(e_gen[:], e_gen[:], e_cps[:BL, :3000],
                                op=ALU.add)
        lnv = konst.tile([BL, 3000], F32, tag="cps_flat")
        nc.scalar.activation(lnv[:], e_gen[:], AF.Ln)
        nc.vector.tensor_scalar(out=lnv[:], in0=lnv[:], scalar1=lse[:, :1],
                                scalar2=None, op0=ALU.subtract)
        nc.sync.dma_start(out_ap[:, 0:3000], lnv[:])

        # ---- OOV scatter-logsumexp into slots V..VOOV ----
        w_oov = sb.tile([BL, Tb], F32, tag="w_oov")
        nc.vector.scalar_tensor_tensor(
            out=w_oov[:], in0=e_cps[:BL, V:VTB], scalar=rtot[:, :1],
            in1=oovf_s[:], op0=ALU.mult, op1=ALU.mult)
        tpw = transpose_cols(w_oov[:BL, :Tb], Tb)
        wT = konst.tile([128, BL], F32, tag="wT")
        nc.vector.tensor_copy(wT[:], tpw[:128, :BL])
        for g in range(4):
            pso = pp_sm.tile([NI, 512], F32, tag="ps_sm")
            for s in range(4):
                b = g * 4 + s
                m2 = sb.tile([128, 4], F32R, tag="m2")
                nc.vector.scalar_tensor_tensor(
                    out=m2[:], in0=iota4f[:], scalar=khiT_s[:, b:b + 1],
                    in1=wT[:, b:b + 1].to_broadcast([128, 4]),
                    op0=ALU.is_equal, op1=ALU.mult)
                lo2 = sb.tile([128, 128], F32R, tag="lo")
                nc.vector.tensor_scalar(
                    out=lo2[:], in0=iota128f[:], scalar1=kloT_s[:, b:b + 1],
                    scalar2=None, op0=ALU.is_equal)
                nc.tensor.matmul(pso[:4, s * 100:(s + 1) * 100],
                                 lhsT=m2[:].bitcast(F32R),
                                 rhs=lo2[:, :100].bitcast(F32R),
                                 start=True, stop=True)
            gtz = sb.tile([4, 400], mybir.dt.uint32, tag="gtz")
            nc.vector.tensor_scalar(out=gtz[:], in0=pso[:4, :400], scalar1=0.0,
                                    scalar2=None, op0=ALU.is_gt)
            mx = sb.tile([4, 400], F32, tag="mx")
            nc.vector.tensor_scalar(out=mx[:], in0=pso[:4, :400],
                                    scalar1=1e-38, scalar2=None, op0=ALU.max)
            lnn = sb.tile([4, 400], F32, tag="lnn")
            nc.scalar.activation(lnn[:], mx[:], AF.Ln)
            res = sb.tile([4, 400], F32, tag="res")
            nc.vector.memset(res[:], NEG)
            nc.vector.copy_predicated(res[:], gtz[:], lnn[:])
            for s in range(4):
                b = g * 4 + s
                nc.sync.dma_start(out_ap[b:b + 1, 3000:3400],
                                  res[:4, s * 100:(s + 1) * 100])


def prep_inputs(inputs):
    """Full inputs -> list of 8 per-core in_maps (host shard/cast/transpose)."""
    f32 = np.float32
    h0 = np.asarray(inputs["dec_last_h"], f32)[0]
    emb_t = np.asarray(inputs["emb_table"], f32)
    attn_W = np.asarray(inputs["attn_W"], f32)
    attn_b = np.asarray(inputs["attn_b"], f32)
    v_w = np.asarray(inputs["v_w"], f32)
    wc_w = np.asarray(inputs["Wcopy_w"], f32)
    wc_b = np.asarray(inputs["Wcopy_b"], f32)
    wg_w = np.asarray(inputs["Wgen_w"], f32)
    wg_b = np.asarray(inputs["Wgen_b"], f32)
    wih = np.asarray(inputs["gru_W_ih"], f32)
    whh = np.asarray(inputs["gru_W_hh"], f32)
    bih = np.asarray(inputs["gru_b_ih"], f32)
    bhh = np.asarray(inputs["gru_b_hh"], f32)
    db = np.asarray(inputs["db"], f32)
    dlw = np.asarray(inputs["dec_last_w"]).astype(np.int64)[:, 0]
    nounk = np.asarray(inputs["bspn_nounk"]).astype(np.int64)

    aT = attn_W.T  # [1024, 512]
    wihT = np.zeros((17 * 128, 1536), f32)
    wihT[:2080] = wih.T

    def chunk17(a):  # [17*128, X] -> [128, 17*X]
        x = a.shape[1]
        return np.ascontiguousarray(
            a.reshape(17, 128, x).transpose(1, 0, 2).reshape(128, 17 * x))

    shared = {
        "w1t": _chunked(np.ascontiguousarray(aT[:512])).astype(_nbf),
        "w2t": _chunked(np.ascontiguousarray(aT[512:])).astype(_nbf),
        "wct": _chunked(np.ascontiguousarray(wc_w.T)).astype(_nbf),
        "wgent": _chunked(np.ascontiguousarray(wg_w.T)).astype(_nbf),
        "wihrz": chunk17(np.ascontiguousarray(wihT[:, :1024])).astype(_nbf),
        "wihn": chunk17(np.ascontiguousarray(wihT[:, 1024:])).astype(_nbf),
        "whht": _chunked(np.ascontiguousarray(whh.T)).astype(_nbf),
        "attnb": attn_b.reshape(1, 512).astype(_nbf),
        "wgenb": wg_b.reshape(1, 3000).astype(_nbf),
        "gbrz": (bih[:1024] + bhh[:1024]).reshape(1, 1024).astype(_nbf),
        "gbinn": bih[1024:].reshape(1, 512).astype(_nbf),
        "gbhn": bhh[1024:].reshape(1, 512).astype(_nbf),
        "wcb": np.ascontiguousarray(wc_b.reshape(4, 128).T).astype(f32),
        "vq16": _chunked(np.repeat(v_w.reshape(512, 1), BL, axis=1)
                         ).astype(_nbf),
    }

    enc_full = {"u": np.asarray(inputs["usdx_h"], f32),
                "b": np.asarray(inputs["bspn_h"], f32),
                "p": np.asarray(inputs["pvaspn_h"], f32)}
    ids_full = {"u": np.asarray(inputs["usdx_ids"]),
                "b": np.asarray(inputs["bspn_ids"]),
                "p": np.asarray(inputs["pvaspn_ids"])}

    tloc = np.arange(Tb)
    col_full = np.where(nounk < V, nounk, V + tloc[None, :])
    k_full = np.clip(nounk - V, 0, NOOV - 1)
    oov_full = (nounk >= V).astype(f32)

    in_maps = []
    for c in range(NCORES):
        sl = slice(c * BL, (c + 1) * BL)
        m = dict(shared)
        for key, T in ENCS:
            e = enc_full[key][sl]
            eT = e.transpose(2, 0, 1).reshape(512, BL * T)
            m[key + "T"] = _chunked(eT).astype(_nbf)
            m[key + "N"] = np.ascontiguousarray(
                e.reshape(BL * T, 512)).astype(_nbf)
            msk = np.where(ids_full[key][sl] == 0, NEG, 0.0).astype(f32)
            m["ms" + key] = np.broadcast_to(
                msk.reshape(1, BL * T), (BL, BL * T)).astype(f32)
            if key == "b":
                m["mbT"] = np.ascontiguousarray(msk.T).astype(f32)
        h0c = h0[sl]
        m["h0t"] = _chunked(np.ascontiguousarray(h0c.T)).astype(_nbf)
        m["h0f"] = h0c.astype(f32)
        m["embt"] = _chunked(np.ascontiguousarray(emb_t[dlw[sl]].T)
                             ).astype(_nbf)
        m["dbt"] = np.ascontiguousarray(db[sl].T).astype(_nbf)
        m["colhiT"] = np.ascontiguousarray((col_full[sl] // 128).T).astype(f32)
        m["colloT"] = np.ascontiguousarray((col_full[sl] % 128).T).astype(f32)
        m["khiT"] = np.ascontiguousarray((k_full[sl] // 100).T).astype(f32)
        m["kloT"] = np.ascontiguousarray((k_full[sl] % 100).T).astype(f32)
        m["oovf"] = np.ascontiguousarray(oov_full[sl]).astype(f32)
        in_maps.append(m)
    return in_maps


_nc_cache = None


_nc_key = None


def get_program():
    global _nc_cache, _nc_key
    key = (DEBUG, PHASE)
    if _nc_cache is None or _nc_key != key:
        _nc_cache = build_program()
        _nc_key = key
    return _nc_cache


def run(inputs, trace=False, tmpdir=None):
    nc = get_program()
    in_maps = prep_inputs(inputs)
    res = run_bass_kernel_spmd(nc, in_maps, list(range(NCORES)), trace=trace,
                               tmpdir=tmpdir)
    out = np.concatenate([res.results[c]["out"][:, None, :]
                          for c in range(NCORES)], axis=0)
    return np.ascontiguousarray(out.astype(np.float32)), res


def kernel(**inputs) -> np.ndarray:
    out, _ = run(inputs)
    return out


# revision 14
# speedup vs baseline: 1.0251x; 1.0251x over previous
"""ActSpanDecoder step on 8 Trainium2 NeuronCores.

Strategy: data-parallel over batch (16 rows/core), identical SPMD program on
all 8 cores (no collectives). The [B,Tb,V+Tb] one-hot scatter matrix is never
read: the copy-score scatter is reconstructed on-device from bspn_nounk via a
factorized (25x128) one-hot matmul, which accumulates duplicate indices in
PSUM exactly like the reference einsum. Matmul inputs are bf16 (fp32
accumulate); softmax / log-space math stays fp32.
"""

import sys

sys.path.insert(0, "/opt/trn_rl_repo")

import numpy as np
import ml_dtypes

import concourse.bass as bass
import concourse.tile as tile
from concourse import bacc, mybir
from concourse.bass_utils import run_bass_kernel_spmd
from concourse.masks import make_identity

BF16 = mybir.dt.bfloat16
F32 = mybir.dt.float32
F32R = mybir.dt.float32r
I32 = mybir.dt.int32
AF = mybir.ActivationFunctionType
ALU = mybir.AluOpType
AX = mybir.AxisListType

NCORES = 8
B, Tu, Tb, Tp = 128, 256, 128, 64
BL = B // NCORES  # 16
H, E, V, VOOV, PTR = 512, 512, 3000, 3400, 32
NEG = -1e20
HC = 4
NI = 25          # 25*128 = 3200 >= V+Tb = 3128
NIP = 26         # padded matmul M (even partition count for PSUM reads)
VP = NI * 128
NOOV = VOOV - V  # 400
VTB = V + Tb

_nbf = ml_dtypes.bfloat16

DEBUG = False
PHASE = 99  # bisection: stop after phase N

ENCS = [("u", Tu), ("b", Tb), ("p", Tp)]


def _chunked(a):
    """[512, X] -> [128, 4*X] with h-chunk c at cols [c*X:(c+1)*X]."""
    h, x = a.shape
    assert h == 4 * 128
    return np.ascontiguousarray(
        a.reshape(4, 128, x).transpose(1, 0, 2).reshape(128, 4 * x)
    )


def build_program():
    nc = bacc.Bacc("TRN2", target_bir_lowering=False, debug=False,
                   num_devices=NCORES)

    def din(name, shape, dt=BF16):
        return nc.dram_tensor(name, list(shape), dt, kind="ExternalInput").ap()

    ins = {}
    for name, shape, dt in [
        ("uT", (128, 4 * BL * Tu), BF16), ("bT", (128, 4 * BL * Tb), BF16),
        ("pT", (128, 4 * BL * Tp), BF16),
        ("uN", (128, 32 * 512), BF16), ("bN", (128, 16 * 512), BF16),
        ("pN", (128, 8 * 512), BF16),
        ("w1t", (128, 4 * 512), BF16), ("w2t", (128, 4 * 512), BF16),
        ("wct", (128, 4 * 512), BF16), ("wgent", (128, 4 * 3000), BF16),
        ("wihrz", (128, 17 * 1024), BF16), ("wihn", (128, 17 * 512), BF16),
        ("whht", (128, 4 * 1536), BF16),
        ("attnb", (1, 512), BF16), ("wgenb", (1, 3000), BF16),
        ("gbrz", (1, 1024), BF16), ("gbinn", (1, 512), BF16),
        ("gbhn", (1, 512), BF16),
        ("h0t", (128, 4 * BL), BF16), ("h0f", (BL, 512), F32),
        ("embt", (128, 4 * BL), BF16), ("dbt", (PTR, BL), BF16),
        ("vq16", (128, 4 * BL), BF16), ("wcb", (128, 4), F32),
        ("msu", (BL, BL * Tu), BF16), ("msb", (BL, BL * Tb), BF16),
        ("msp", (BL, BL * Tp), BF16), ("mbT", (128, BL), F32),
        ("colhiT", (128, BL), F32), ("colloT", (128, BL), F32),
        ("khiT", (128, BL), F32), ("kloT", (128, BL), F32),
        ("oovf", (BL, Tb), F32),
    ]:
        ins[name] = din(name, shape, dt)

    out_ap = nc.dram_tensor("out", [BL, VOOV], F32, kind="ExternalOutput").ap()

    dbg = {}
    if DEBUG:
        for name, shape in [
            ("dq", (BL, 512)), ("dctxu", (BL, 512)), ("dctxb", (BL, 512)),
            ("dctxp", (BL, 512)), ("dhnew", (BL, 512)), ("dcprawT", (128, BL)),
            ("dcps", (BL, VP)), ("dlse", (BL, 1)),
        ]:
            dbg[name] = nc.dram_tensor(name, list(shape), F32,
                                       kind="ExternalOutput").ap()

    with tile.TileContext(nc) as tc:
        _emit(tc, nc, ins, out_ap, dbg)

    nc.compile()
    return nc


def _emit(tc, nc, ins, out_ap, dbg):
    from contextlib import ExitStack

    ctx = ExitStack()
    with ctx:
        konst = ctx.enter_context(tc.tile_pool(name="konst", bufs=1))
        sb = ctx.enter_context(tc.tile_pool(name="sb", bufs=1))
        enc_pool = ctx.enter_context(tc.tile_pool(name="encn", bufs=3))
        et_pool = ctx.enter_context(tc.tile_pool(name="et", bufs=6))
        wstream = ctx.enter_context(tc.tile_pool(name="wstream", bufs=3))
        pp_s = ctx.enter_context(tc.tile_pool(name="pps", bufs=3, space="PSUM"))
        pp_sm = ctx.enter_context(tc.tile_pool(name="ppsm", bufs=2, space="PSUM"))
        pp_tr = ctx.enter_context(tc.tile_pool(name="pptr", bufs=1, space="PSUM"))

        # ---- constants ----
        ident = konst.tile([128, 128], F32, tag="ident")
        make_identity(nc, ident[:])
        ones16 = konst.tile([1, BL], BF16, tag="ones16")
        nc.vector.memset(ones16[:], 1.0)

        def load(name, dt=BF16, pool=konst):
            ap = ins[name]
            t = pool.tile(list(ap.shape), dt, tag="k_" + name)
            nc.sync.dma_start(t[:], ap[:])
            return t

        w1t_s = load("w1t")
        w2t_s = load("w2t")
        wct_s = load("wct")
        attnb_s = load("attnb")
        wgenb_s = load("wgenb")
        gbrz_s = load("gbrz")
        gbinn_s = load("gbinn")
        gbhn_s = load("gbhn")
        h0t_s = load("h0t")
        h0f_s = load("h0f", F32)
        embt_s = load("embt")
        dbt_s = load("dbt")
        vq16_s = load("vq16")
        wcb_s = load("wcb", F32)
        maskst_s = {"u": load("msu"), "b": load("msb"),
                    "p": load("msp")}
        maskbT_s = load("mbT", F32)
        colhiT_s = load("colhiT", F32)
        colloT_s = load("colloT", F32)
        khiT_s = load("khiT", F32)
        kloT_s = load("kloT", F32)
        oovf_s = load("oovf", F32)
        encT = {"u": ins["uT"], "b": ins["bT"], "p": ins["pT"]}
        encN = {"u": ins["uN"], "b": ins["bN"], "p": ins["pN"]}
        etin_pool = ctx.enter_context(tc.tile_pool(name="etin", bufs=6))

        iota25i = konst.tile([128, NIP], I32, tag="iota25i")
        nc.gpsimd.iota(iota25i[:], pattern=[[1, NIP]], base=0,
                       channel_multiplier=0)
        iota25f = konst.tile([128, NIP], F32, tag="iota25f")
        nc.vector.tensor_copy(iota25f[:], iota25i[:])
        iota128i = konst.tile([128, 128], I32, tag="iota128i")
        nc.gpsimd.iota(iota128i[:], pattern=[[1, 128]], base=0,
                       channel_multiplier=0)
        iota128f = konst.tile([128, 128], F32, tag="iota128f")
        nc.vector.tensor_copy(iota128f[:], iota128i[:])
        iota4f = konst.tile([128, 4], F32, tag="iota4f")
        nc.vector.tensor_copy(iota4f[:], iota128i[:, :4])


        def _pad_out():
            zout = sb.tile([BL, VOOV], F32, tag="zout")
            nc.vector.memset(zout[:], 0.0)
            nc.sync.dma_start(out_ap[:], zout[:])

        def transpose_cols(src_ap, nf):
            """src [BL, nf] -> psum [nf, BL] f32 (nf <= 128)."""
            tp = pp_tr.tile([128, BL], F32, tag="tp")
            nc.tensor.transpose(tp[:nf, :BL], src_ap, ident[:BL, :BL])
            return tp

        # ---- q = h0 @ W1.T + attn_b ----
        qp = pp_sm.tile([NI, 512], F32, tag="ps_sm")
        for hc in range(HC):
            nc.tensor.matmul(qp[:BL, :512],
                             lhsT=h0t_s[:, hc * BL:(hc + 1) * BL],
                             rhs=w1t_s[:, hc * 512:(hc + 1) * 512],
                             start=(hc == 0), stop=False)
        nc.tensor.matmul(qp[:BL, :512], lhsT=ones16[:1, :], rhs=attnb_s[:1, :],
                         start=False, stop=True)
        q_sb = konst.tile([BL, 512], F32, tag="q_sb")
        nc.vector.tensor_copy(q_sb[:], qp[:BL, :512])
        if DEBUG:
            nc.sync.dma_start(dbg["dq"][:], q_sb[:])
        qt_sb = konst.tile([128, 4 * BL], F32, tag="qt_sb")
        for hc in range(HC):
            tp = transpose_cols(q_sb[:BL, hc * 128:(hc + 1) * 128], 128)
            nc.vector.tensor_copy(qt_sb[:, hc * BL:(hc + 1) * BL],
                                  tp[:128, :BL])

        if PHASE < 2:
            _pad_out()
            return
        # ---- attention over the three encoders ----
        ctx_sb = {}
        encs_run = ENCS[:1] if PHASE == 2 else ENCS
        for key, T in encs_run:
            cols = BL * T
            ntiles = cols // 512
            nseg = 512 // T
            eT = encT[key]
            nchunks = cols // 128
            abig = sb.tile([128, nchunks * BL], BF16, tag="abig_" + key)
            nc.vector.memset(abig[:], 0.0)
            asump = sb.tile([BL, BL], F32, tag="asump_" + key)
            etw = min(2048, cols)
            etin = None
            for nt in range(ntiles):
                if nt % (etw // 512) == 0:
                    etin = []
                    for hic in range(HC):
                        ei = etin_pool.tile([128, etw], BF16, tag="etin")
                        nc.sync.dma_start(
                            ei[:], eT[:, hic * cols + nt * 512:
                                      hic * cols + nt * 512 + etw])
                        etin.append(ei)
                off = (nt % (etw // 512)) * 512
                ets = []
                for hoc in range(HC):
                    ps = pp_s.tile([128, 512], F32, tag="ps_s")
                    for hic in range(HC):
                        nc.tensor.matmul(
                            ps[:],
                            lhsT=w2t_s[:, hic * 512 + hoc * 128:
                                       hic * 512 + (hoc + 1) * 128],
                            rhs=etin[hic][:, off:off + 512],
                            start=(hic == 0), stop=(hic == HC - 1))
                    et = et_pool.tile([128, 512], BF16, tag="et")
                    for s in range(nseg):
                        b = nt * nseg + s
                        nc.scalar.activation(
                            et[:, s * T:(s + 1) * T], ps[:, s * T:(s + 1) * T],
                            AF.Tanh,
                            bias=qt_sb[:, hoc * BL + b:hoc * BL + b + 1])
                    ets.append(et)
                strip = pp_sm.tile([NI, 512], F32, tag="ps_sm")
                for hoc in range(HC):
                    nc.tensor.matmul(strip[:BL, :512],
                                     lhsT=vq16_s[:, hoc * BL:(hoc + 1) * BL],
                                     rhs=ets[hoc][:], start=(hoc == 0),
                                     stop=(hoc == HC - 1))
                # masked scores in replicated-row strip layout
                astr = sb.tile([BL, 512], F32, tag="astr")
                nc.vector.tensor_tensor(
                    astr[:], strip[:BL, :512],
                    maskst_s[key][:, nt * 512:(nt + 1) * 512], op=ALU.add)
                aex = sb.tile([BL, 512], F32, tag="aex")
                for s in range(nseg):
                    b = nt * nseg + s
                    nc.scalar.activation(aex[:, s * T:(s + 1) * T],
                                         astr[:, s * T:(s + 1) * T], AF.Exp,
                                         accum_out=asump[:, b:b + 1])
                # A^T columns via PE transpose of 128-col blocks
                for blk in range(4):
                    tp = transpose_cols(aex[:BL, blk * 128:(blk + 1) * 128],
                                        128)
                    gtok = nt * 512 + blk * 128  # global (b,t) token index
                    c = gtok // 128
                    if T >= 128:
                        b = gtok // T
                        nc.vector.tensor_copy(
                            abig[:128, c * BL + b:c * BL + b + 1],
                            tp[:128, 0:1])
                    else:  # pv: the 128-token chunk spans two b rows
                        for h in range(2):
                            b = (gtok + h * 64) // T
                            nc.vector.tensor_copy(
                                abig[h * 64:(h + 1) * 64,
                                     c * BL + b:c * BL + b + 1],
                                tp[h * 64:(h + 1) * 64, 0:1])
            # per-b softmax sums: diagonal of asump via identity-masked reduce
            asum = sb.tile([BL, 1], F32, tag="asum_" + key)
            djunk = sb.tile([BL, BL], F32, tag="djunk")
            nc.vector.tensor_tensor(djunk[:], asump[:], ident[:BL, :BL],
                                    op=ALU.mult)
            nc.vector.tensor_reduce(asum[:], djunk[:], axis=AX.X, op=ALU.add)
            rec = sb.tile([BL, 1], F32, tag="rec_" + key)
            nc.vector.reciprocal(rec[:], asum[:])
            # ctx = (A @ enc) / sum
            ctxp = pp_sm.tile([NI, 512], F32, tag="ps_sm")
            encn = None
            for c in range(nchunks):
                if c % 4 == 0:
                    encn = enc_pool.tile([128, 2048], BF16, tag="encn")
                    nc.sync.dma_start(
                        encn[:], encN[key][:, c * 512:(c + 4) * 512])
                nc.tensor.matmul(ctxp[:BL, :512],
                                 lhsT=abig[:128, c * BL:(c + 1) * BL],
                                 rhs=encn[:, (c % 4) * 512:(c % 4 + 1) * 512],
                                 start=(c == 0), stop=(c == nchunks - 1))
            cx = konst.tile([BL, 512], F32, tag="ctx_" + key)
            nc.vector.tensor_scalar(out=cx[:], in0=ctxp[:BL, :512],
                                    scalar1=rec[:, :1], scalar2=None,
                                    op0=ALU.mult)
            ctx_sb[key] = cx
            if DEBUG:
                nc.sync.dma_start(dbg["dctx" + key][:], cx[:])

        if PHASE < 4:
            _pad_out()
            return
        # ---- xT assembly: emb | ctx_u | ctx_b | ctx_p | db ----
        xT = konst.tile([128, 17 * BL], BF16, tag="xT")
        nc.vector.memset(xT[:], 0.0)
        nc.vector.tensor_copy(xT[:, 0:4 * BL], embt_s[:])
        for i, key in enumerate(["u", "b", "p"]):
            for hc in range(HC):
                tp = transpose_cols(ctx_sb[key][:BL, hc * 128:(hc + 1) * 128],
                                    128)
                col = (4 + 4 * i + hc) * BL
                nc.vector.tensor_copy(xT[:, col:col + BL], tp[:128, :BL])
        nc.vector.tensor_copy(xT[:PTR, 16 * BL:17 * BL], dbt_s[:])

        if PHASE < 5:
            _pad_out()
            return
        # ---- GRU (pass 1: r,z with ih+hh fused in PSUM) ----
        ps_r = pp_sm.tile([NI, 512], F32, tag="ps_sm")
        ps_z = pp_sm.tile([NI, 512], F32, tag="ps_sm")
        for k in range(17):
            wrz = wstream.tile([128, 1024], BF16, tag="wrz")
            nc.sync.dma_start(wrz[:], ins["wihrz"][:, k * 1024:(k + 1) * 1024])
            lhs = xT[:, k * BL:(k + 1) * BL]
            nc.tensor.matmul(ps_r[:BL, :512], lhsT=lhs, rhs=wrz[:, 0:512],
                             start=(k == 0), stop=False)
            nc.tensor.matmul(ps_z[:BL, :512], lhsT=lhs, rhs=wrz[:, 512:1024],
                             start=(k == 0), stop=False)
        for hc in range(HC):
            whrz = wstream.tile([128, 1024], BF16, tag="wrz")
            nc.sync.dma_start(whrz[:],
                              ins["whht"][:, hc * 1536:hc * 1536 + 1024])
            lhs = h0t_s[:, hc * BL:(hc + 1) * BL]
            nc.tensor.matmul(ps_r[:BL, :512], lhsT=lhs, rhs=whrz[:, 0:512],
                             start=False, stop=False)
            nc.tensor.matmul(ps_z[:BL, :512], lhsT=lhs, rhs=whrz[:, 512:1024],
                             start=False, stop=False)
        nc.tensor.matmul(ps_r[:BL, :512], lhsT=ones16[:1, :],
                         rhs=gbrz_s[:1, 0:512], start=False, stop=True)
        nc.tensor.matmul(ps_z[:BL, :512], lhsT=ones16[:1, :],
                         rhs=gbrz_s[:1, 512:1024], start=False, stop=True)
        r_sb = sb.tile([BL, 512], F32, tag="r_sb")
        z_sb = sb.tile([BL, 512], F32, tag="z_sb")
        nc.scalar.activation(r_sb[:], ps_r[:BL, :512], AF.Sigmoid)
        nc.scalar.activation(z_sb[:], ps_z[:BL, :512], AF.Sigmoid)

        # ---- GRU (pass 2: inn, hn) ----
        ps_inn = pp_sm.tile([NI, 512], F32, tag="ps_sm")
        ps_hn = pp_sm.tile([NI, 512], F32, tag="ps_sm")
        for k in range(17):
            wn = wstream.tile([128, 512], BF16, tag="wn")
            nc.sync.dma_start(wn[:], ins["wihn"][:, k * 512:(k + 1) * 512])
            nc.tensor.matmul(ps_inn[:BL, :512], lhsT=xT[:, k * BL:(k + 1) * BL],
                             rhs=wn[:], start=(k == 0), stop=False)
        nc.tensor.matmul(ps_inn[:BL, :512], lhsT=ones16[:1, :],
                         rhs=gbinn_s[:1, :], start=False, stop=True)
        for hc in range(HC):
            whn = wstream.tile([128, 512], BF16, tag="wn")
            nc.sync.dma_start(whn[:],
                              ins["whht"][:, hc * 1536 + 1024:hc * 1536 + 1536])
            nc.tensor.matmul(ps_hn[:BL, :512],
                             lhsT=h0t_s[:, hc * BL:(hc + 1) * BL],
                             rhs=whn[:], start=(hc == 0), stop=False)
        nc.tensor.matmul(ps_hn[:BL, :512], lhsT=ones16[:1, :],
                         rhs=gbhn_s[:1, :], start=False, stop=True)

        rhn = sb.tile([BL, 512], F32, tag="rhn")
        nc.vector.tensor_tensor(rhn[:], r_sb[:], ps_hn[:BL, :512], op=ALU.mult)
        npre = sb.tile([BL, 512], F32, tag="npre")
        nc.vector.tensor_tensor(npre[:], rhn[:], ps_inn[:BL, :512], op=ALU.add)
        n_sb = sb.tile([BL, 512], F32, tag="n_sb")
        nc.scalar.activation(n_sb[:], npre[:], AF.Tanh)
        t1 = sb.tile([BL, 512], F32, tag="rhn")
        nc.vector.tensor_tensor(t1[:], h0f_s[:], n_sb[:], op=ALU.subtract)
        t2 = sb.tile([BL, 512], F32, tag="npre")
        nc.vector.tensor_tensor(t2[:], z_sb[:], t1[:], op=ALU.mult)
        hnew = konst.tile([BL, 512], F32, tag="hnew")
        nc.vector.tensor_tensor(hnew[:], t2[:], n_sb[:], op=ALU.add)
        if DEBUG:
            nc.sync.dma_start(dbg["dhnew"][:], hnew[:])
        hnT = konst.tile([128, 4 * BL], BF16, tag="hnT")
        for hc in range(HC):
            tp = transpose_cols(hnew[:BL, hc * 128:(hc + 1) * 128], 128)
            nc.vector.tensor_copy(hnT[:, hc * BL:(hc + 1) * BL], tp[:128, :BL])

        if PHASE < 6:
            _pad_out()
            return
        # ---- copy scores: cp_raw[b,t] = tanh(bspn Wc.T + bc) . hnew + mask ----
        bcols = BL * Tb
        cprawT = konst.tile([128, BL], F32, tag="cprawT")
        for nt in range(4):
            if nt == 0:
                etin = []
                for hic in range(HC):
                    ei = etin_pool.tile([128, 2048], BF16, tag="etin")
                    nc.sync.dma_start(ei[:], encT["b"][:, hic * bcols:
                                                       hic * bcols + 2048])
                    etin.append(ei)
            off2 = nt * 512
            cpts = []
            for hoc in range(HC):
                ps = pp_s.tile([128, 512], F32, tag="ps_s")
                for hic in range(HC):
                    nc.tensor.matmul(
                        ps[:],
                        lhsT=wct_s[:, hic * 512 + hoc * 128:
                                   hic * 512 + (hoc + 1) * 128],
                        rhs=etin[hic][:, off2:off2 + 512],
                        start=(hic == 0), stop=(hic == HC - 1))
                cpt = et_pool.tile([128, 512], BF16, tag="et")
                nc.scalar.activation(cpt[:], ps[:], AF.Tanh,
                                     bias=wcb_s[:, hoc:hoc + 1])
                cpts.append(cpt)
            pscr = pp_sm.tile([NI, 512], F32, tag="ps_sm")
            for hoc in range(HC):
                nc.tensor.matmul(pscr[:BL, :512],
                                 lhsT=hnT[:, hoc * BL:(hoc + 1) * BL],
                                 rhs=cpts[hoc][:], start=(hoc == 0),
                                 stop=(hoc == HC - 1))
            stsb = sb.tile([BL, 512], F32, tag="stripsb")
            nc.any.tensor_copy(stsb[:], pscr[:BL, :512])
            for s in range(4):
                b = nt * 4 + s
                tpc = transpose_cols(stsb[:BL, s * 128:(s + 1) * 128], 128)
                nc.vector.tensor_copy(cprawT[:, b:b + 1], tpc[:128, b:b + 1])
        nc.vector.tensor_tensor(cprawT[:], cprawT[:], maskbT_s[:], op=ALU.add)
        if DEBUG:
            nc.sync.dma_start(dbg["dcprawT"][:], cprawT[:])

        if PHASE < 7:
            _pad_out()
            return
        # ---- factorized scatter: cps[col[t]] += cp_raw[t] ----
        scat_sb = konst.tile([26, 4 * 512], F32, tag="k_egen")
        for g in range(4):
            psc = pp_sm.tile([26, 512], F32, tag="ps_sm")
            for s in range(4):
                b = g * 4 + s
                m1 = sb.tile([128, NIP], F32R, tag="m1")
                nc.vector.scalar_tensor_tensor(
                    out=m1[:], in0=iota25f[:], scalar=colhiT_s[:, b:b + 1],
                    in1=cprawT[:, b:b + 1].to_broadcast([128, NIP]),
                    op0=ALU.is_equal, op1=ALU.mult)
                lo = sb.tile([128, 128], F32R, tag="lo")
                nc.vector.tensor_scalar(
                    out=lo[:], in0=iota128f[:], scalar1=colloT_s[:, b:b + 1],
                    scalar2=None, op0=ALU.is_equal)
                nc.tensor.matmul(psc[:NIP, s * 128:(s + 1) * 128],
                                 lhsT=m1[:], rhs=lo[:],
                                 start=True, stop=True)
            nc.vector.tensor_copy(scat_sb[:26, g * 512:(g + 1) * 512],
                                  psc[:26, :512])
        cps_flat = konst.tile([BL, VP], F32, tag="cps_flat")
        for b in range(BL):
            g, s = b // 4, b % 4
            nc.sync.dma_start(
                cps_flat[b:b + 1, :],
                scat_sb[:NI, g * 512 + s * 128:g * 512 + (s + 1) * 128])
        if DEBUG:
            nc.sync.dma_start(dbg["dcps"][:], cps_flat[:])
        e_cps = konst.tile([BL, VTB], F32, tag="e_cps")
        scs = sb.tile([BL, 1], F32, tag="scs")
        nc.scalar.activation(e_cps[:], cps_flat[:BL, :VTB], AF.Exp,
                             accum_out=scs[:])

        if PHASE < 8:
            _pad_out()
            return
        # ---- gen = exp(hnew @ Wgen.T + b) ----
        e_gen = konst.tile([BL, 3000], F32, tag="k_egen")
        sgp = sb.tile([BL, 6], F32, tag="sgp")
        nts = [512] * 5 + [440]
        for i, n in enumerate(nts):
            off = i * 512
            pg = pp_sm.tile([NI, 512], F32, tag="ps_sm")
            for hc in range(HC):
                wg = wstream.tile([128, 512], BF16, tag="wn")
                nc.sync.dma_start(wg[:, :n],
                                  ins["wgent"][:, hc * 3000 + off:
                                               hc * 3000 + off + n])
                nc.tensor.matmul(
                    pg[:BL, :n], lhsT=hnT[:, hc * BL:(hc + 1) * BL],
                    rhs=wg[:, :n], start=(hc == 0), stop=False)
            nc.tensor.matmul(pg[:BL, :n], lhsT=ones16[:1, :],
                             rhs=wgenb_s[:1, off:off + n], start=False,
                             stop=True)
            nc.scalar.activation(e_gen[:, off:off + n], pg[:BL, :n], AF.Exp,
                                 accum_out=sgp[:, i:i + 1])
        sg = sb.tile([BL, 1], F32, tag="sg")
        nc.vector.tensor_reduce(sg[:], sgp[:], axis=AX.X, op=ALU.add)

        # ---- normalization ----
        stot = sb.tile([BL, 1], F32, tag="stot")
        nc.vector.tensor_tensor(stot[:], sg[:], scs[:], op=ALU.add)
        lse = sb.tile([BL, 1], F32, tag="lse")
        nc.scalar.activation(lse[:], stot[:], AF.Ln)
        rtot = sb.tile([BL, 1], F32, tag="rtot")
        nc.vector.reciprocal(rtot[:], stot[:])
        if DEBUG:
            nc.sync.dma_start(dbg["dlse"][:], lse[:])

        # total[:, :V] = ln(e_gen + e_cps[:, :V]) - lse
        nc.vector.tensor_tensor(e_gen[:], e_gen[:], e_cps[:BL, :3000],
                                op=ALU.add)
        lnv = konst.tile([BL, 3000], F32, tag="cps_flat")
        nc.scalar.activation(lnv[:], e_gen[:], AF.Ln)
        nc.vector.tensor_scalar(out=lnv[:], in0=lnv[:], scalar1=lse[:, :1],
                                scalar2=None, op0=ALU.subtract)
        nc.sync.dma_start(out_ap[:, 0:3000], lnv[:])

        # ---- OOV scatter-logsumexp into slots V..VOOV ----
        w_oov = sb.tile([BL, Tb], F32, tag="w_oov")
        nc.vector.scalar_tensor_tensor(
            out=w_oov[:], in0=e_cps[:BL, V:VTB], scalar=rtot[:, :1],
            in1=oovf_s[:], op0=ALU.mult, op1=ALU.mult)
        tpw = transpose_cols(w_oov[:BL, :Tb], Tb)
        wT = konst.tile([128, BL], F32, tag="wT")
        nc.vector.tensor_copy(wT[:], tpw[:128, :BL])
        for g in range(4):
            pso = pp_sm.tile([NI, 512], F32, tag="ps_sm")
            for s in range(4):
                b = g * 4 + s
                m2 = sb.tile([128, 4], F32R, tag="m2")
                nc.vector.scalar_tensor_tensor(
                    out=m2[:], in0=iota4f[:], scalar=khiT_s[:, b:b + 1],
                    in1=wT[:, b:b + 1].to_broadcast([128, 4]),
                    op0=ALU.is_equal, op1=ALU.mult)
                lo2 = sb.tile([128, 128], F32R, tag="lo")
                nc.vector.tensor_scalar(
                    out=lo2[:], in0=iota128f[:], scalar1=kloT_s[:, b:b + 1],
                    scalar2=None, op0=ALU.is_equal)
                nc.tensor.matmul(pso[:4, s * 100:(s + 1) * 100],
                                 lhsT=m2[:].bitcast(F32R),
                                 rhs=lo2[:, :100].bitcast(F32R),
                                 start=True, stop=True)
            gtz = sb.tile([4, 400], mybir.dt.uint32, tag="gtz")
            nc.vector.tensor_scalar(out=gtz[:], in0=pso[:4, :400], scalar1=0.0,
                                    scalar2=None, op0=ALU.is_gt)
            mx = sb.tile([4, 400], F32, tag="mx")
            nc.vector.tensor_scalar(out=mx[:], in0=pso[:4, :400],
                                    scalar1=1e-38, scalar2=None, op0=ALU.max)
            lnn = sb.tile([4, 400], F32, tag="lnn")
            nc.scalar.activation(lnn[:], mx[:], AF.Ln)
            res = sb.tile([4, 400], F32, tag="res")
            nc.vector.memset(res[:], NEG)
            nc.vector.copy_predicated(res[:], gtz[:], lnn[:])
            for s in range(4):
                b = g * 4 + s
                nc.sync.dma_start(out_ap[b:b + 1, 3000:3400],
                                  res[:4, s * 100:(s + 1) * 100])


def prep_inputs(inputs):
    """Full inputs -> list of 8 per-core in_maps (host shard/cast/transpose)."""
    f32 = np.float32
    h0 = np.asarray(inputs["dec_last_h"], f32)[0]
    emb_t = np.asarray(inputs["emb_table"], f32)
    attn_W = np.asarray(inputs["attn_W"], f32)
    attn_b = np.asarray(inputs["attn_b"], f32)
    v_w = np.asarray(inputs["v_w"], f32)
    wc_w = np.asarray(inputs["Wcopy_w"], f32)
    wc_b = np.asarray(inputs["Wcopy_b"], f32)
    wg_w = np.asarray(inputs["Wgen_w"], f32)
    wg_b = np.asarray(inputs["Wgen_b"], f32)
    wih = np.asarray(inputs["gru_W_ih"], f32)
    whh = np.asarray(inputs["gru_W_hh"], f32)
    bih = np.asarray(inputs["gru_b_ih"], f32)
    bhh = np.asarray(inputs["gru_b_hh"], f32)
    db = np.asarray(inputs["db"], f32)
    dlw = np.asarray(inputs["dec_last_w"]).astype(np.int64)[:, 0]
    nounk = np.asarray(inputs["bspn_nounk"]).astype(np.int64)

    aT = attn_W.T  # [1024, 512]
    wihT = np.zeros((17 * 128, 1536), f32)
    wihT[:2080] = wih.T

    def chunk17(a):  # [17*128, X] -> [128, 17*X]
        x = a.shape[1]
        return np.ascontiguousarray(
            a.reshape(17, 128, x).transpose(1, 0, 2).reshape(128, 17 * x))

    shared = {
        "w1t": _chunked(np.ascontiguousarray(aT[:512])).astype(_nbf),
        "w2t": _chunked(np.ascontiguousarray(aT[512:])).astype(_nbf),
        "wct": _chunked(np.ascontiguousarray(wc_w.T)).astype(_nbf),
        "wgent": _chunked(np.ascontiguousarray(wg_w.T)).astype(_nbf),
        "wihrz": chunk17(np.ascontiguousarray(wihT[:, :1024])).astype(_nbf),
        "wihn": chunk17(np.ascontiguousarray(wihT[:, 1024:])).astype(_nbf),
        "whht": _chunked(np.ascontiguousarray(whh.T)).astype(_nbf),
        "attnb": attn_b.reshape(1, 512).astype(_nbf),
        "wgenb": wg_b.reshape(1, 3000).astype(_nbf),
        "gbrz": (bih[:1024] + bhh[:1024]).reshape(1, 1024).astype(_nbf),
        "gbinn": bih[1024:].reshape(1, 512).astype(_nbf),
        "gbhn": bhh[1024:].reshape(1, 512).astype(_nbf),
        "wcb": np.ascontiguousarray(wc_b.reshape(4, 128).T).astype(f32),
        "vq16": _chunked(np.repeat(v_w.reshape(512, 1), BL, axis=1)
                         ).astype(_nbf),
    }

    enc_full = {"u": np.asarray(inputs["usdx_h"], f32),
                "b": np.asarray(inputs["bspn_h"], f32),
                "p": np.asarray(inputs["pvaspn_h"], f32)}
    ids_full = {"u": np.asarray(inputs["usdx_ids"]),
                "b": np.asarray(inputs["bspn_ids"]),
                "p": np.asarray(inputs["pvaspn_ids"])}

    tloc = np.arange(Tb)
    col_full = np.where(nounk < V, nounk, V + tloc[None, :])
    k_full = np.clip(nounk - V, 0, NOOV - 1)
    oov_full = (nounk >= V).astype(f32)

    in_maps = []
    for c in range(NCORES):
        sl = slice(c * BL, (c + 1) * BL)
        m = dict(shared)
        for key, T in ENCS:
            e = enc_full[key][sl]
            eT = e.transpose(2, 0, 1).reshape(512, BL * T)
            m[key + "T"] = _chunked(eT).astype(_nbf)
            nch = (BL * T) // 128
            m[key + "N"] = np.ascontiguousarray(
                e.reshape(nch, 128, 512).transpose(1, 0, 2).reshape(
                    128, nch * 512)).astype(_nbf)
            msk = np.where(ids_full[key][sl] == 0, NEG, 0.0).astype(f32)
            m["ms" + key] = np.broadcast_to(
                msk.reshape(1, BL * T), (BL, BL * T)).astype(_nbf)
            if key == "b":
                m["mbT"] = np.ascontiguousarray(msk.T).astype(f32)
        h0c = h0[sl]
        m["h0t"] = _chunked(np.ascontiguousarray(h0c.T)).astype(_nbf)
        m["h0f"] = h0c.astype(f32)
        m["embt"] = _chunked(np.ascontiguousarray(emb_t[dlw[sl]].T)
                             ).astype(_nbf)
        m["dbt"] = np.ascontiguousarray(db[sl].T).astype(_nbf)
        m["colhiT"] = np.ascontiguousarray((col_full[sl] // 128).T).astype(f32)
        m["colloT"] = np.ascontiguousarray((col_full[sl] % 128).T).astype(f32)
        m["khiT"] = np.ascontiguousarray((k_full[sl] // 100).T).astype(f32)
        m["kloT"] = np.ascontiguousarray((k_full[sl] % 100).T).astype(f32)
        m["oovf"] = np.ascontiguousarray(oov_full[sl]).astype(f32)
        in_maps.append(m)
    return in_maps


_nc_cache = None


_nc_key = None


def get_program():
    global _nc_cache, _nc_key
    key = (DEBUG, PHASE)
    if _nc_cache is None or _nc_key != key:
        _nc_cache = build_program()
        _nc_key = key
    return _nc_cache


def run(inputs, trace=False, tmpdir=None):
    nc = get_program()
    in_maps = prep_inputs(inputs)
    res = run_bass_kernel_spmd(nc, in_maps, list(range(NCORES)), trace=trace,
                               tmpdir=tmpdir)
    out = np.concatenate([res.results[c]["out"][:, None, :]
                          for c in range(NCORES)], axis=0)
    return np.ascontiguousarray(out.astype(np.float32)), res


def kernel(**inputs) -> np.ndarray:
    out, _ = run(inputs)
    return out


# revision 16
# speedup vs baseline: 1.2049x; 1.1754x over previous
"""ActSpanDecoder step on 8 Trainium2 NeuronCores.

Strategy: data-parallel over batch (16 rows/core), identical SPMD program on
all 8 cores (no collectives). The [B,Tb,V+Tb] one-hot scatter matrix is never
read: the copy-score scatter is reconstructed on-device from bspn_nounk via a
factorized (25x128) one-hot matmul, which accumulates duplicate indices in
PSUM exactly like the reference einsum. Matmul inputs are bf16 (fp32
accumulate); softmax / log-space math stays fp32.
"""

import sys

sys.path.insert(0, "/opt/trn_rl_repo")

import numpy as np
import ml_dtypes

import concourse.bass as bass
import concourse.tile as tile
from concourse import bacc, mybir
from concourse.bass_utils import run_bass_kernel_spmd
from concourse.masks import make_identity

BF16 = mybir.dt.bfloat16
F32 = mybir.dt.float32
F32R = mybir.dt.float32r
I32 = mybir.dt.int32
AF = mybir.ActivationFunctionType
ALU = mybir.AluOpType
AX = mybir.AxisListType

NCORES = 8
B, Tu, Tb, Tp = 128, 256, 128, 64
BL = B // NCORES  # 16
H, E, V, VOOV, PTR = 512, 512, 3000, 3400, 32
NEG = -1e20
HC = 4
NI = 25          # 25*128 = 3200 >= V+Tb = 3128
NIP = 26         # padded matmul M (even partition count for PSUM reads)
VP = NI * 128
NOOV = VOOV - V  # 400
VTB = V + Tb

_nbf = ml_dtypes.bfloat16

DEBUG = False
PHASE = 99  # bisection: stop after phase N

ENCS = [("u", Tu), ("b", Tb), ("p", Tp)]


def _chunked(a):
    """[512, X] -> [128, 4*X] with h-chunk c at cols [c*X:(c+1)*X]."""
    h, x = a.shape
    assert h == 4 * 128
    return np.ascontiguousarray(
        a.reshape(4, 128, x).transpose(1, 0, 2).reshape(128, 4 * x)
    )


def build_program():
    nc = bacc.Bacc("TRN2", target_bir_lowering=False, debug=False,
                   num_devices=NCORES)

    def din(name, shape, dt=BF16):
        return nc.dram_tensor(name, list(shape), dt, kind="ExternalInput").ap()

    ins = {}
    for name, shape, dt in [
        ("uT", (128, 4 * BL * Tu), BF16), ("bT", (128, 4 * BL * Tb), BF16),
        ("pT", (128, 4 * BL * Tp), BF16),
        ("uN", (128, 32 * 512), BF16), ("bN", (128, 16 * 512), BF16),
        ("pN", (128, 8 * 512), BF16),
        ("w1t", (128, 4 * 512), BF16), ("w2t", (128, 4 * 512), BF16),
        ("wct", (128, 4 * 512), BF16), ("wgent", (128, 4 * 3000), BF16),
        ("wihrz", (128, 17 * 1024), BF16), ("wihn", (128, 17 * 512), BF16),
        ("whht", (128, 4 * 1536), BF16),
        ("attnb", (1, 512), BF16), ("wgenb", (1, 3000), BF16),
        ("gbrz", (1, 1024), BF16), ("gbinn", (1, 512), BF16),
        ("gbhn", (1, 512), BF16),
        ("h0t", (128, 4 * BL), BF16), ("h0f", (BL, 512), F32),
        ("embt", (128, 4 * BL), BF16), ("dbt", (PTR, BL), BF16),
        ("vq16", (128, 4 * BL), BF16), ("wcb", (128, 4), F32),
        ("msu", (1, BL * Tu), BF16), ("msb", (1, BL * Tb), BF16),
        ("msp", (1, BL * Tp), BF16), ("mbT", (128, BL), F32),
        ("colhiT", (128, BL), F32), ("colloT", (128, BL), F32),
        ("khiT", (128, BL), F32), ("kloT", (128, BL), F32),
        ("oovf", (BL, Tb), F32),
    ]:
        ins[name] = din(name, shape, dt)

    out_ap = nc.dram_tensor("out", [BL, VOOV], F32, kind="ExternalOutput").ap()

    dbg = {}
    if DEBUG:
        for name, shape in [
            ("dq", (BL, 512)), ("dctxu", (BL, 512)), ("dctxb", (BL, 512)),
            ("dctxp", (BL, 512)), ("dhnew", (BL, 512)), ("dcprawT", (128, BL)),
            ("dcps", (BL, VP)), ("dlse", (BL, 1)),
        ]:
            dbg[name] = nc.dram_tensor(name, list(shape), F32,
                                       kind="ExternalOutput").ap()

    with tile.TileContext(nc) as tc:
        _emit(tc, nc, ins, out_ap, dbg)

    nc.compile()
    return nc


def _emit(tc, nc, ins, out_ap, dbg):
    from contextlib import ExitStack

    ctx = ExitStack()
    with ctx:
        konst = ctx.enter_context(tc.tile_pool(name="konst", bufs=1))
        sb = ctx.enter_context(tc.tile_pool(name="sb", bufs=1))
        enc_pool = ctx.enter_context(tc.tile_pool(name="encn", bufs=3))
        et_pool = ctx.enter_context(tc.tile_pool(name="et", bufs=6))
        wstream = ctx.enter_context(tc.tile_pool(name="wstream", bufs=3))
        pp_s = ctx.enter_context(tc.tile_pool(name="pps", bufs=3, space="PSUM"))
        pp_sm = ctx.enter_context(tc.tile_pool(name="ppsm", bufs=2, space="PSUM"))
        pp_tr = ctx.enter_context(tc.tile_pool(name="pptr", bufs=2, space="PSUM"))

        # ---- constants ----
        ident = konst.tile([128, 128], F32, tag="ident")
        make_identity(nc, ident[:])
        ones16 = konst.tile([1, BL], BF16, tag="ones16")
        nc.vector.memset(ones16[:], 1.0)

        def load(name, dt=BF16, pool=konst):
            ap = ins[name]
            t = pool.tile(list(ap.shape), dt, tag="k_" + name)
            nc.sync.dma_start(t[:], ap[:])
            return t

        w1t_s = load("w1t")
        w2t_s = load("w2t")
        wct_s = load("wct")
        attnb_s = load("attnb")
        wgenb_s = load("wgenb")
        gbrz_s = load("gbrz")
        gbinn_s = load("gbinn")
        gbhn_s = load("gbhn")
        h0t_s = load("h0t")
        h0f_s = load("h0f", F32)
        embt_s = load("embt")
        dbt_s = load("dbt")
        vq16_s = load("vq16")
        wcb_s = load("wcb", F32)
        maskst_s = {"u": load("msu"), "b": load("msb"),
                    "p": load("msp")}  # [1, BL*T] additive rows
        maskbT_s = load("mbT", F32)
        colhiT_s = load("colhiT", F32)
        colloT_s = load("colloT", F32)
        khiT_s = load("khiT", F32)
        kloT_s = load("kloT", F32)
        oovf_s = load("oovf", F32)
        encT = {"u": ins["uT"], "b": ins["bT"], "p": ins["pT"]}
        encN = {"u": ins["uN"], "b": ins["bN"], "p": ins["pN"]}
        etin_pool = ctx.enter_context(tc.tile_pool(name="etin", bufs=6))

        iota25i = konst.tile([128, NIP], I32, tag="iota25i")
        nc.gpsimd.iota(iota25i[:], pattern=[[1, NIP]], base=0,
                       channel_multiplier=0)
        iota25f = konst.tile([128, NIP], F32, tag="iota25f")
        nc.vector.tensor_copy(iota25f[:], iota25i[:])
        iota128i = konst.tile([128, 128], I32, tag="iota128i")
        nc.gpsimd.iota(iota128i[:], pattern=[[1, 128]], base=0,
                       channel_multiplier=0)
        iota128f = konst.tile([128, 128], F32, tag="iota128f")
        nc.vector.tensor_copy(iota128f[:], iota128i[:])
        iota4f = konst.tile([128, 4], F32, tag="iota4f")
        nc.vector.tensor_copy(iota4f[:], iota128i[:, :4])


        def _pad_out():
            zout = sb.tile([BL, VOOV], F32, tag="zout")
            nc.vector.memset(zout[:], 0.0)
            nc.sync.dma_start(out_ap[:], zout[:])

        def transpose_cols(src_ap, nf):
            """src [BL, nf] -> psum [nf, BL] f32 (nf <= 128)."""
            tp = pp_tr.tile([128, BL], F32, tag="tp")
            nc.tensor.transpose(tp[:nf, :BL], src_ap, ident[:BL, :BL])
            return tp

        # ---- q = h0 @ W1.T + attn_b ----
        qp = pp_sm.tile([NI, 512], F32, tag="ps_sm")
        for hc in range(HC):
            nc.tensor.matmul(qp[:BL, :512],
                             lhsT=h0t_s[:, hc * BL:(hc + 1) * BL],
                             rhs=w1t_s[:, hc * 512:(hc + 1) * 512],
                             start=(hc == 0), stop=False)
        nc.tensor.matmul(qp[:BL, :512], lhsT=ones16[:1, :], rhs=attnb_s[:1, :],
                         start=False, stop=True)
        q_sb = konst.tile([BL, 512], F32, tag="q_sb")
        nc.vector.tensor_copy(q_sb[:], qp[:BL, :512])
        if DEBUG:
            nc.sync.dma_start(dbg["dq"][:], q_sb[:])
        qt_sb = konst.tile([128, 4 * BL], F32, tag="qt_sb")
        for hc in range(HC):
            tp = transpose_cols(q_sb[:BL, hc * 128:(hc + 1) * 128], 128)
            nc.vector.tensor_copy(qt_sb[:, hc * BL:(hc + 1) * BL],
                                  tp[:128, :BL])

        if PHASE < 2:
            _pad_out()
            return
        # ---- attention over the three encoders ----
        ctx_sb = {}
        encs_run = ENCS[:1] if PHASE == 2 else ENCS
        for key, T in encs_run:
            cols = BL * T
            ntiles = cols // 512
            nseg = 512 // T
            eT = encT[key]
            nchunks = cols // 128
            abig = sb.tile([128, nchunks * BL], BF16, tag="abig_" + key)
            nc.vector.memset(abig[:], 0.0)
            asump = sb.tile([BL, BL], F32, tag="asump_" + key)
            etw = min(2048, cols)
            etin = None
            for nt in range(ntiles):
                if nt % (etw // 512) == 0:
                    etin = []
                    for hic in range(HC):
                        ei = etin_pool.tile([128, etw], BF16, tag="etin")
                        nc.gpsimd.dma_start(
                            ei[:], eT[:, hic * cols + nt * 512:
                                      hic * cols + nt * 512 + etw])
                        etin.append(ei)
                off = (nt % (etw // 512)) * 512
                ets = []
                for hoc in range(HC):
                    ps = pp_s.tile([128, 512], F32, tag="ps_s")
                    for hic in range(HC):
                        nc.tensor.matmul(
                            ps[:],
                            lhsT=w2t_s[:, hic * 512 + hoc * 128:
                                       hic * 512 + (hoc + 1) * 128],
                            rhs=etin[hic][:, off:off + 512],
                            start=(hic == 0), stop=(hic == HC - 1))
                    et = et_pool.tile([128, 512], BF16, tag="et")
                    for s in range(nseg):
                        b = nt * nseg + s
                        nc.scalar.activation(
                            et[:, s * T:(s + 1) * T], ps[:, s * T:(s + 1) * T],
                            AF.Tanh,
                            bias=qt_sb[:, hoc * BL + b:hoc * BL + b + 1])
                    ets.append(et)
                strip = pp_sm.tile([NI, 512], F32, tag="ps_sm")
                for hoc in range(HC):
                    nc.tensor.matmul(strip[:BL, :512],
                                     lhsT=vq16_s[:, hoc * BL:(hoc + 1) * BL],
                                     rhs=ets[hoc][:], start=(hoc == 0),
                                     stop=False)
                # fold the pad mask in as a rank-1 matmul (ones x mask-row)
                nc.tensor.matmul(strip[:BL, :512], lhsT=ones16[:1, :],
                                 rhs=maskst_s[key][:1, nt * 512:(nt + 1) * 512],
                                 start=False, stop=True)
                aex = sb.tile([BL, 512], F32, tag="aex")
                for s in range(nseg):
                    b = nt * nseg + s
                    nc.scalar.activation(aex[:, s * T:(s + 1) * T],
                                         strip[:BL, s * T:(s + 1) * T], AF.Exp,
                                         accum_out=asump[:, b:b + 1])
                # A^T columns via PE transpose of 128-col blocks
                for blk in range(4):
                    tp = transpose_cols(aex[:BL, blk * 128:(blk + 1) * 128],
                                        128)
                    gtok = nt * 512 + blk * 128  # global (b,t) token index
                    c = gtok // 128
                    if T >= 128:
                        b = gtok // T
                        nc.vector.tensor_copy(
                            abig[:128, c * BL + b:c * BL + b + 1],
                            tp[:128, 0:1])
                    else:  # pv: the 128-token chunk spans two b rows
                        for h in range(2):
                            b = (gtok + h * 64) // T
                            nc.vector.tensor_copy(
                                abig[h * 64:(h + 1) * 64,
                                     c * BL + b:c * BL + b + 1],
                                tp[h * 64:(h + 1) * 64, 0:1])
            # per-b softmax sums: diagonal of asump via identity-masked reduce
            asum = sb.tile([BL, 1], F32, tag="asum_" + key)
            djunk = sb.tile([BL, BL], F32, tag="djunk")
            nc.vector.tensor_tensor(djunk[:], asump[:], ident[:BL, :BL],
                                    op=ALU.mult)
            nc.vector.tensor_reduce(asum[:], djunk[:], axis=AX.X, op=ALU.add)
            rec = sb.tile([BL, 1], F32, tag="rec_" + key)
            nc.vector.reciprocal(rec[:], asum[:])
            # ctx = (A @ enc) / sum
            ctxp = pp_sm.tile([NI, 512], F32, tag="ps_sm")
            encn = None
            for c in range(nchunks):
                if c % 4 == 0:
                    encn = enc_pool.tile([128, 2048], BF16, tag="encn")
                    nc.gpsimd.dma_start(
                        encn[:], encN[key][:, c * 512:(c + 4) * 512])
                nc.tensor.matmul(ctxp[:BL, :512],
                                 lhsT=abig[:128, c * BL:(c + 1) * BL],
                                 rhs=encn[:, (c % 4) * 512:(c % 4 + 1) * 512],
                                 start=(c == 0), stop=(c == nchunks - 1))
            cx = konst.tile([BL, 512], F32, tag="ctx_" + key)
            nc.vector.tensor_scalar(out=cx[:], in0=ctxp[:BL, :512],
                                    scalar1=rec[:, :1], scalar2=None,
                                    op0=ALU.mult)
            ctx_sb[key] = cx
            if DEBUG:
                nc.sync.dma_start(dbg["dctx" + key][:], cx[:])

        if PHASE < 4:
            _pad_out()
            return
        # ---- xT assembly: emb | ctx_u | ctx_b | ctx_p | db ----
        xT = konst.tile([128, 17 * BL], BF16, tag="xT")
        nc.vector.memset(xT[:], 0.0)
        nc.vector.tensor_copy(xT[:, 0:4 * BL], embt_s[:])
        for i, key in enumerate(["u", "b", "p"]):
            for hc in range(HC):
                tp = transpose_cols(ctx_sb[key][:BL, hc * 128:(hc + 1) * 128],
                                    128)
                col = (4 + 4 * i + hc) * BL
                nc.vector.tensor_copy(xT[:, col:col + BL], tp[:128, :BL])
        nc.vector.tensor_copy(xT[:PTR, 16 * BL:17 * BL], dbt_s[:])

        if PHASE < 5:
            _pad_out()
            return
        # ---- GRU (pass 1: r,z with ih+hh fused in PSUM) ----
        ps_r = pp_sm.tile([NI, 512], F32, tag="ps_sm")
        ps_z = pp_sm.tile([NI, 512], F32, tag="ps_sm")
        for k in range(17):
            if k % 2 == 0:
                w2 = min(2048, (17 - k) * 1024)
                wrz = wstream.tile([128, 2048], BF16, tag="wrz")
                nc.gpsimd.dma_start(wrz[:, :w2],
                                    ins["wihrz"][:, k * 1024:k * 1024 + w2])
            ko = (k % 2) * 1024
            lhs = xT[:, k * BL:(k + 1) * BL]
            nc.tensor.matmul(ps_r[:BL, :512], lhsT=lhs,
                             rhs=wrz[:, ko:ko + 512],
                             start=(k == 0), stop=False)
            nc.tensor.matmul(ps_z[:BL, :512], lhsT=lhs,
                             rhs=wrz[:, ko + 512:ko + 1024],
                             start=(k == 0), stop=False)
        for hc in range(HC):
            whrz = wstream.tile([128, 2048], BF16, tag="wrz")
            nc.gpsimd.dma_start(whrz[:, :1024],
                                ins["whht"][:, hc * 1536:hc * 1536 + 1024])
            lhs = h0t_s[:, hc * BL:(hc + 1) * BL]
            nc.tensor.matmul(ps_r[:BL, :512], lhsT=lhs, rhs=whrz[:, 0:512],
                             start=False, stop=False)
            nc.tensor.matmul(ps_z[:BL, :512], lhsT=lhs,
                             rhs=whrz[:, 512:1024], start=False, stop=False)
        nc.tensor.matmul(ps_r[:BL, :512], lhsT=ones16[:1, :],
                         rhs=gbrz_s[:1, 0:512], start=False, stop=True)
        nc.tensor.matmul(ps_z[:BL, :512], lhsT=ones16[:1, :],
                         rhs=gbrz_s[:1, 512:1024], start=False, stop=True)
        r_sb = sb.tile([BL, 512], F32, tag="r_sb")
        z_sb = sb.tile([BL, 512], F32, tag="z_sb")
        nc.scalar.activation(r_sb[:], ps_r[:BL, :512], AF.Sigmoid)
        nc.scalar.activation(z_sb[:], ps_z[:BL, :512], AF.Sigmoid)

        # ---- GRU (pass 2: inn, hn) ----
        ps_inn = pp_sm.tile([NI, 512], F32, tag="ps_sm")
        ps_hn = pp_sm.tile([NI, 512], F32, tag="ps_sm")
        for k in range(17):
            if k % 2 == 0:
                w2 = min(1024, (17 - k) * 512)
                wn = wstream.tile([128, 1024], BF16, tag="wn")
                nc.gpsimd.dma_start(wn[:, :w2],
                                    ins["wihn"][:, k * 512:k * 512 + w2])
            ko = (k % 2) * 512
            nc.tensor.matmul(ps_inn[:BL, :512], lhsT=xT[:, k * BL:(k + 1) * BL],
                             rhs=wn[:, ko:ko + 512], start=(k == 0),
                             stop=False)
        nc.tensor.matmul(ps_inn[:BL, :512], lhsT=ones16[:1, :],
                         rhs=gbinn_s[:1, :], start=False, stop=True)
        for hc in range(HC):
            whn = wstream.tile([128, 1024], BF16, tag="wn")
            nc.gpsimd.dma_start(whn[:, :512],
                                ins["whht"][:, hc * 1536 + 1024:hc * 1536 + 1536])
            nc.tensor.matmul(ps_hn[:BL, :512],
                             lhsT=h0t_s[:, hc * BL:(hc + 1) * BL],
                             rhs=whn[:, :512], start=(hc == 0), stop=False)
        nc.tensor.matmul(ps_hn[:BL, :512], lhsT=ones16[:1, :],
                         rhs=gbhn_s[:1, :], start=False, stop=True)

        rhn = sb.tile([BL, 512], F32, tag="rhn")
        nc.vector.tensor_tensor(rhn[:], r_sb[:], ps_hn[:BL, :512], op=ALU.mult)
        npre = sb.tile([BL, 512], F32, tag="npre")
        nc.vector.tensor_tensor(npre[:], rhn[:], ps_inn[:BL, :512], op=ALU.add)
        n_sb = sb.tile([BL, 512], F32, tag="n_sb")
        nc.scalar.activation(n_sb[:], npre[:], AF.Tanh)
        t1 = sb.tile([BL, 512], F32, tag="rhn")
        nc.vector.tensor_tensor(t1[:], h0f_s[:], n_sb[:], op=ALU.subtract)
        t2 = sb.tile([BL, 512], F32, tag="npre")
        nc.vector.tensor_tensor(t2[:], z_sb[:], t1[:], op=ALU.mult)
        hnew = konst.tile([BL, 512], F32, tag="hnew")
        nc.vector.tensor_tensor(hnew[:], t2[:], n_sb[:], op=ALU.add)
        if DEBUG:
            nc.sync.dma_start(dbg["dhnew"][:], hnew[:])
        hnT = konst.tile([128, 4 * BL], BF16, tag="hnT")
        for hc in range(HC):
            tp = transpose_cols(hnew[:BL, hc * 128:(hc + 1) * 128], 128)
            nc.vector.tensor_copy(hnT[:, hc * BL:(hc + 1) * BL], tp[:128, :BL])

        if PHASE < 6:
            _pad_out()
            return
        # ---- copy scores: cp_raw[b,t] = tanh(bspn Wc.T + bc) . hnew + mask ----
        bcols = BL * Tb
        cprawT = konst.tile([128, BL], F32, tag="cprawT")
        for nt in range(4):
            if nt == 0:
                etin = []
                for hic in range(HC):
                    ei = etin_pool.tile([128, 2048], BF16, tag="etin")
                    nc.gpsimd.dma_start(ei[:], encT["b"][:, hic * bcols:
                                                       hic * bcols + 2048])
                    etin.append(ei)
            off2 = nt * 512
            cpts = []
            for hoc in range(HC):
                ps = pp_s.tile([128, 512], F32, tag="ps_s")
                for hic in range(HC):
                    nc.tensor.matmul(
                        ps[:],
                        lhsT=wct_s[:, hic * 512 + hoc * 128:
                                   hic * 512 + (hoc + 1) * 128],
                        rhs=etin[hic][:, off2:off2 + 512],
                        start=(hic == 0), stop=(hic == HC - 1))
                cpt = et_pool.tile([128, 512], BF16, tag="et")
                nc.scalar.activation(cpt[:], ps[:], AF.Tanh,
                                     bias=wcb_s[:, hoc:hoc + 1])
                cpts.append(cpt)
            pscr = pp_sm.tile([NI, 512], F32, tag="ps_sm")
            for hoc in range(HC):
                nc.tensor.matmul(pscr[:BL, :512],
                                 lhsT=hnT[:, hoc * BL:(hoc + 1) * BL],
                                 rhs=cpts[hoc][:], start=(hoc == 0),
                                 stop=(hoc == HC - 1))
            stsb = sb.tile([BL, 512], F32, tag="stripsb")
            nc.any.tensor_copy(stsb[:], pscr[:BL, :512])
            for s in range(4):
                b = nt * 4 + s
                tpc = transpose_cols(stsb[:BL, s * 128:(s + 1) * 128], 128)
                nc.vector.tensor_copy(cprawT[:, b:b + 1], tpc[:128, b:b + 1])
        nc.vector.tensor_tensor(cprawT[:], cprawT[:], maskbT_s[:], op=ALU.add)
        if DEBUG:
            nc.sync.dma_start(dbg["dcprawT"][:], cprawT[:])

        if PHASE < 7:
            _pad_out()
            return
        # ---- factorized scatter: cps[col[t]] += cp_raw[t] ----
        scat_sb = konst.tile([26, 4 * 512], F32, tag="k_egen")
        for g in range(4):
            psc = pp_sm.tile([26, 512], F32, tag="ps_sm")
            for s in range(4):
                b = g * 4 + s
                m1 = sb.tile([128, NIP], F32R, tag="m1%d" % (b % 2))
                nc.vector.scalar_tensor_tensor(
                    out=m1[:], in0=iota25f[:], scalar=colhiT_s[:, b:b + 1],
                    in1=cprawT[:, b:b + 1].to_broadcast([128, NIP]),
                    op0=ALU.is_equal, op1=ALU.mult)
                lo = sb.tile([128, 128], F32R, tag="lo%d" % (b % 2))
                nc.vector.tensor_scalar(
                    out=lo[:], in0=iota128f[:], scalar1=colloT_s[:, b:b + 1],
                    scalar2=None, op0=ALU.is_equal)
                nc.tensor.matmul(psc[:NIP, s * 128:(s + 1) * 128],
                                 lhsT=m1[:], rhs=lo[:],
                                 start=True, stop=True)
            nc.vector.tensor_copy(scat_sb[:26, g * 512:(g + 1) * 512],
                                  psc[:26, :512])
        cps_flat = konst.tile([BL, VP], F32, tag="cps_flat")
        for b in range(BL):
            g, s = b // 4, b % 4
            nc.sync.dma_start(
                cps_flat[b:b + 1, :],
                scat_sb[:NI, g * 512 + s * 128:g * 512 + (s + 1) * 128])
        if DEBUG:
            nc.sync.dma_start(dbg["dcps"][:], cps_flat[:])
        e_cps = konst.tile([BL, VTB], F32, tag="e_cps")
        scs = sb.tile([BL, 1], F32, tag="scs")
        nc.scalar.activation(e_cps[:], cps_flat[:BL, :VTB], AF.Exp,
                             accum_out=scs[:])

        if PHASE < 8:
            _pad_out()
            return
        # ---- gen = exp(hnew @ Wgen.T + b) ----
        e_gen = konst.tile([BL, 3000], F32, tag="k_egen")
        sgp = sb.tile([BL, 6], F32, tag="sgp")
        nts = [512] * 5 + [440]
        for i, n in enumerate(nts):
            off = i * 512
            pg = pp_sm.tile([NI, 512], F32, tag="ps_sm")
            for hc in range(HC):
                wg = wstream.tile([128, 1024], BF16, tag="wn")
                nc.gpsimd.dma_start(wg[:, :n],
                                  ins["wgent"][:, hc * 3000 + off:
                                               hc * 3000 + off + n])
                nc.tensor.matmul(
                    pg[:BL, :n], lhsT=hnT[:, hc * BL:(hc + 1) * BL],
                    rhs=wg[:, :n], start=(hc == 0), stop=False)
            nc.tensor.matmul(pg[:BL, :n], lhsT=ones16[:1, :],
                             rhs=wgenb_s[:1, off:off + n], start=False,
                             stop=True)
            nc.scalar.activation(e_gen[:, off:off + n], pg[:BL, :n], AF.Exp,
                                 accum_out=sgp[:, i:i + 1])
        sg = sb.tile([BL, 1], F32, tag="sg")
        nc.vector.tensor_reduce(sg[:], sgp[:], axis=AX.X, op=ALU.add)

        # ---- normalization ----
        stot = sb.tile([BL, 1], F32, tag="stot")
        nc.vector.tensor_tensor(stot[:], sg[:], scs[:], op=ALU.add)
        rtot = sb.tile([BL, 1], F32, tag="rtot")
        nc.vector.reciprocal(rtot[:], stot[:])
        if DEBUG:
            lse = sb.tile([BL, 1], F32, tag="lse")
            nc.scalar.activation(lse[:], stot[:], AF.Ln)
            nc.sync.dma_start(dbg["dlse"][:], lse[:])

        # total[:, :V] = ln((e_gen + e_cps[:, :V]) * r)   [= ln(.) - lse]
        nc.vector.tensor_tensor(e_gen[:], e_gen[:], e_cps[:BL, :3000],
                                op=ALU.add)
        lnv = konst.tile([BL, 3000], F32, tag="cps_flat")
        nc.scalar.activation(lnv[:], e_gen[:], AF.Ln, scale=rtot[:, :1])
        nc.sync.dma_start(out_ap[:, 0:3000], lnv[:])

        # ---- OOV scatter-logsumexp into slots V..VOOV ----
        w_oov = sb.tile([BL, Tb], F32, tag="w_oov")
        nc.vector.scalar_tensor_tensor(
            out=w_oov[:], in0=e_cps[:BL, V:VTB], scalar=rtot[:, :1],
            in1=oovf_s[:], op0=ALU.mult, op1=ALU.mult)
        tpw = transpose_cols(w_oov[:BL, :Tb], Tb)
        wT = konst.tile([128, BL], F32, tag="wT")
        nc.vector.tensor_copy(wT[:], tpw[:128, :BL])
        for g in range(4):
            pso = pp_sm.tile([NI, 512], F32, tag="ps_sm")
            for s in range(4):
                b = g * 4 + s
                m2 = sb.tile([128, 4], F32R, tag="m2%d" % (b % 2))
                nc.vector.scalar_tensor_tensor(
                    out=m2[:], in0=iota4f[:], scalar=khiT_s[:, b:b + 1],
                    in1=wT[:, b:b + 1].to_broadcast([128, 4]),
                    op0=ALU.is_equal, op1=ALU.mult)
                lo2 = sb.tile([128, 128], F32R, tag="lo%d" % (b % 2))
                nc.vector.tensor_scalar(
                    out=lo2[:], in0=iota128f[:], scalar1=kloT_s[:, b:b + 1],
                    scalar2=None, op0=ALU.is_equal)
                nc.tensor.matmul(pso[:4, s * 100:(s + 1) * 100],
                                 lhsT=m2[:].bitcast(F32R),
                                 rhs=lo2[:, :100].bitcast(F32R),
                                 start=True, stop=True)
            gtz = sb.tile([4, 400], mybir.dt.uint32, tag="gtz%d" % (g % 2))
            nc.vector.tensor_scalar(out=gtz[:], in0=pso[:4, :400], scalar1=0.0,
                                    scalar2=None, op0=ALU.is_gt)
            lnn = sb.tile([4, 400], F32, tag="lnn%d" % (g % 2))
            nc.scalar.activation(lnn[:], pso[:4, :400], AF.Ln)
            res = sb.tile([4, 400], F32, tag="res%d" % (g % 2))
            nc.vector.memset(res[:], NEG)
            nc.vector.copy_predicated(res[:], gtz[:], lnn[:])
            for s in range(4):
                b = g * 4 + s
                nc.sync.dma_start(out_ap[b:b + 1, 3000:3400],
                                  res[:4, s * 100:(s + 1) * 100])


def prep_inputs(inputs):
    """Full inputs -> list of 8 per-core in_maps (host shard/cast/transpose)."""
    f32 = np.float32
    h0 = np.asarray(inputs["dec_last_h"], f32)[0]
    emb_t = np.asarray(inputs["emb_table"], f32)
    attn_W = np.asarray(inputs["attn_W"], f32)
    attn_b = np.asarray(inputs["attn_b"], f32)
    v_w = np.asarray(inputs["v_w"], f32)
    wc_w = np.asarray(inputs["Wcopy_w"], f32)
    wc_b = np.asarray(inputs["Wcopy_b"], f32)
    wg_w = np.asarray(inputs["Wgen_w"], f32)
    wg_b = np.asarray(inputs["Wgen_b"], f32)
    wih = np.asarray(inputs["gru_W_ih"], f32)
    whh = np.asarray(inputs["gru_W_hh"], f32)
    bih = np.asarray(inputs["gru_b_ih"], f32)
    bhh = np.asarray(inputs["gru_b_hh"], f32)
    db = np.asarray(inputs["db"], f32)
    dlw = np.asarray(inputs["dec_last_w"]).astype(np.int64)[:, 0]
    nounk = np.asarray(inputs["bspn_nounk"]).astype(np.int64)

    aT = attn_W.T  # [1024, 512]
    wihT = np.zeros((17 * 128, 1536), f32)
    wihT[:2080] = wih.T

    def chunk17(a):  # [17*128, X] -> [128, 17*X]
        x = a.shape[1]
        return np.ascontiguousarray(
            a.reshape(17, 128, x).transpose(1, 0, 2).reshape(128, 17 * x))

    shared = {
        "w1t": _chunked(np.ascontiguousarray(aT[:512])).astype(_nbf),
        "w2t": _chunked(np.ascontiguousarray(aT[512:])).astype(_nbf),
        "wct": _chunked(np.ascontiguousarray(wc_w.T)).astype(_nbf),
        "wgent": _chunked(np.ascontiguousarray(wg_w.T)).astype(_nbf),
        "wihrz": chunk17(np.ascontiguousarray(wihT[:, :1024])).astype(_nbf),
        "wihn": chunk17(np.ascontiguousarray(wihT[:, 1024:])).astype(_nbf),
        "whht": _chunked(np.ascontiguousarray(whh.T)).astype(_nbf),
        "attnb": attn_b.reshape(1, 512).astype(_nbf),
        "wgenb": wg_b.reshape(1, 3000).astype(_nbf),
        "gbrz": (bih[:1024] + bhh[:1024]).reshape(1, 1024).astype(_nbf),
        "gbinn": bih[1024:].reshape(1, 512).astype(_nbf),
        "gbhn": bhh[1024:].reshape(1, 512).astype(_nbf),
        "wcb": np.ascontiguousarray(wc_b.reshape(4, 128).T).astype(f32),
        "vq16": _chunked(np.repeat(v_w.reshape(512, 1), BL, axis=1)
                         ).astype(_nbf),
    }

    enc_full = {"u": np.asarray(inputs["usdx_h"], f32),
                "b": np.asarray(inputs["bspn_h"], f32),
                "p": np.asarray(inputs["pvaspn_h"], f32)}
    ids_full = {"u": np.asarray(inputs["usdx_ids"]),
                "b": np.asarray(inputs["bspn_ids"]),
                "p": np.asarray(inputs["pvaspn_ids"])}

    tloc = np.arange(Tb)
    col_full = np.where(nounk < V, nounk, V + tloc[None, :])
    k_full = np.clip(nounk - V, 0, NOOV - 1)
    oov_full = (nounk >= V).astype(f32)

    in_maps = []
    for c in range(NCORES):
        sl = slice(c * BL, (c + 1) * BL)
        m = dict(shared)
        for key, T in ENCS:
            e = enc_full[key][sl]
            eT = e.transpose(2, 0, 1).reshape(512, BL * T)
            m[key + "T"] = _chunked(eT).astype(_nbf)
            nch = (BL * T) // 128
            m[key + "N"] = np.ascontiguousarray(
                e.reshape(nch, 128, 512).transpose(1, 0, 2).reshape(
                    128, nch * 512)).astype(_nbf)
            msk = np.where(ids_full[key][sl] == 0, NEG, 0.0).astype(f32)
            m["ms" + key] = msk.reshape(1, BL * T).astype(_nbf)
            if key == "b":
                m["mbT"] = np.ascontiguousarray(msk.T).astype(f32)
        h0c = h0[sl]
        m["h0t"] = _chunked(np.ascontiguousarray(h0c.T)).astype(_nbf)
        m["h0f"] = h0c.astype(f32)
        m["embt"] = _chunked(np.ascontiguousarray(emb_t[dlw[sl]].T)
                             ).astype(_nbf)
        m["dbt"] = np.ascontiguousarray(db[sl].T).astype(_nbf)
        m["colhiT"] = np.ascontiguousarray((col_full[sl] // 128).T).astype(f32)
        m["colloT"] = np.ascontiguousarray((col_full[sl] % 128).T).astype(f32)
        m["khiT"] = np.ascontiguousarray((k_full[sl] // 100).T).astype(f32)
        m["kloT"] = np.ascontiguousarray((k_full[sl] % 100).T).astype(f32)
        m["oovf"] = np.ascontiguousarray(oov_full[sl]).astype(f32)
        in_maps.append(m)
    return in_maps


_nc_cache = None


_nc_key = None


def get_program():
    global _nc_cache, _nc_key
    key = (DEBUG, PHASE)
    if _nc_cache is None or _nc_key != key:
        _nc_cache = build_program()
        _nc_key = key
    return _nc_cache


def run(inputs, trace=False, tmpdir=None):
    nc = get_program()
    in_maps = prep_inputs(inputs)
    res = run_bass_kernel_spmd(nc, in_maps, list(range(NCORES)), trace=trace,
                               tmpdir=tmpdir)
    out = np.concatenate([res.results[c]["out"][:, None, :]
                          for c in range(NCORES)], axis=0)
    return np.ascontiguousarray(out.astype(np.float32)), res


def kernel(**inputs) -> np.ndarray:
    out, _ = run(inputs)
    return out
